# revision 17
# baseline (speedup 1.0000x reference)
"""NoFoDifformer Trainium2 kernel — 8-core SPMD.

Sharding (v3): 1280 rows of x/h per core; u column-sharded for GEMM1 and
row-sharded(transposed) for GEMM2; exactly TWO collectives, both AllGathers:
  AG1: h (each core's feat-encoder rows -> full h, f32r) before GEMM1.
  GEMM1  utx[:, cols_i] = u[:, cols_i]^T h : exact per-core a-slice; lhsT =
         h-full tiles (f32r), moving = u_col row-band chunks -> utx^T local.
  z_loc = new_e_loc * utx_loc (token-major f32r).
  AG2: z_loc + local k1v/k2v partials -> full z + all partials (summed
       locally). No AllReduce chain: collectives are the expensive barrier.
  GEMM2  h_fur rows_i = u[rows_i,:] (z): lhsT = z (f32r), moving = uT chunks.
new_e (eig encoding) is computed on host (tiny).
Both big GEMMs run in float32r (PE full-rate fp32, ~1.5e-4 rel); all small
GEMMs plain fp32. LayerNorm/softmax statistics are batched across the 10
token tiles ([128, 10] stat rows) to keep DVE/ACT instruction counts low.
"""
import numpy as np

import concourse.bass as bass
import concourse.tile as tile
from concourse import bacc, mybir, masks
from concourse.bass_utils import run_bass_kernel_spmd

F32 = mybir.dt.float32
F32R = mybir.dt.float32r
AX = mybir.AxisListType
ALU = mybir.AluOpType
ACT = mybir.ActivationFunctionType

NCORES = 8
N = 10000
P = 10240            # padded
R = P // NCORES      # 1280 rows per core
RT = R // 128        # 10 token tiles per core
C = 64
NFEAT = 512
HID = 128
K = 10
NF = 16
OMEGA = 50.0
DMIN = 0.25
EPS = 1e-5
LAM_INIT = 0.8 - 0.6 * float(np.exp(-0.3 * 0))   # layer 0 -> 0.2

AB = P // 128        # 80 a-blocks (z)
PB = P // 128        # 80 n row-blocks of full h
NSUB = 4             # tail psum tag rotation
TCH = [(0, 512), (512, 512), (1024, 256)]   # token chunks of the 1280 rows

_CACHE = {}
LAST_RESULT = None


def _ln_stats(nc, sp, pieces, d, epst, name):
    """Batched LN stats over RT token tiles. pieces: APs [128, RT*ci] whose
    free dim is (t, ci). Returns (m, inv) each [128, RT]."""
    s = sp.tile([128, RT], F32, tag="ln_s", name=f"{name}_s")
    q = sp.tile([128, RT], F32, tag="ln_q", name=f"{name}_q")
    tmp = sp.tile([128, RT], F32, tag="ln_tmp", name=f"{name}_tmp")
    sq = sp.tile([128, RT * C], F32, tag="ln_sq", name=f"{name}_sq", bufs=1)
    for i, x in enumerate(pieces):
        ci = x.shape[1] // RT
        xv = x.rearrange("p (t c) -> p t c", t=RT)
        nc.vector.reduce_sum((s if i == 0 else tmp)[:], xv, axis=AX.X)
        if i > 0:
            nc.vector.tensor_add(s[:], s[:], tmp[:])
        nc.vector.tensor_tensor(sq[:, : RT * ci], x[:], x[:], op=ALU.mult)
        sqv = sq[:, : RT * ci].rearrange("p (t c) -> p t c", t=RT)
        nc.vector.reduce_sum((q if i == 0 else tmp)[:], sqv, axis=AX.X)
        if i > 0:
            nc.vector.tensor_add(q[:], q[:], tmp[:])
    m = sp.tile([128, RT], F32, tag="ln_m", name=f"{name}_m")
    nc.vector.tensor_scalar_mul(m[:], s[:], 1.0 / d)
    nc.vector.tensor_scalar_mul(q[:], q[:], 1.0 / d)
    nc.vector.tensor_tensor(tmp[:], m[:], m[:], op=ALU.mult)
    nc.vector.tensor_sub(q[:], q[:], tmp[:])
    std = sp.tile([128, RT], F32, tag="ln_std", name=f"{name}_std")
    nc.scalar.activation(std[:], q[:], ACT.Sqrt, bias=epst[:, 0:1])
    inv = sp.tile([128, RT], F32, tag="ln_inv", name=f"{name}_inv")
    nc.vector.reciprocal(inv[:], std[:])
    return m, inv


def _ln_norm(nc, x, m1, inv1, g, b, out, scratch):
    """out = (x - m1)*inv1*g + b for one [128, ci] tile (m1/inv1: [128,1])."""
    nc.vector.tensor_scalar(scratch[:], x[:], m1, inv1,
                            op0=ALU.subtract, op1=ALU.mult)
    nc.vector.tensor_tensor(scratch[:], scratch[:], g[:], op=ALU.mult)
    nc.vector.tensor_tensor(out[:], scratch[:], b[:], op=ALU.add)


def build_nc():
    nc = bacc.Bacc("TRN2", target_bir_lowering=False, debug=False, num_devices=NCORES)

    # ---- I/O ----
    u_d = nc.dram_tensor("u", [P, R], F32R, kind="ExternalInput")   # u[:, cols_i]
    ut_d = nc.dram_tensor("ut", [P, R], F32R, kind="ExternalInput")
    xt_d = nc.dram_tensor("xt", [NFEAT, R], F32R, kind="ExternalInput")
    few1_d = nc.dram_tensor("few1", [NFEAT, HID], F32R, kind="ExternalInput")
    few2_d = nc.dram_tensor("few2", [HID, C], F32R, kind="ExternalInput")
    feb1_d = nc.dram_tensor("feb1", [HID, 1], F32, kind="ExternalInput")
    feb2_d = nc.dram_tensor("feb2", [C, 1], F32, kind="ExternalInput")
    newe_d = nc.dram_tensor("newe", [128, RT], F32, kind="ExternalInput")
    mask_d = nc.dram_tensor("mask", [128, RT], F32, kind="ExternalInput")
    wq_d = nc.dram_tensor("wq", [C, 2 * C], F32, kind="ExternalInput")
    wkv_d = nc.dram_tensor("wkv", [C, 3 * C], F32, kind="ExternalInput")
    bq_d = nc.dram_tensor("bq", [C, 2], F32, kind="ExternalInput")
    bkv_d = nc.dram_tensor("bkv", [128, 3 * C], F32, kind="ExternalInput")
    mhgb_d = nc.dram_tensor("mhgb", [128, 2 * C], F32, kind="ExternalInput")
    attgb_d = nc.dram_tensor("attgb", [128, 2 * C], F32, kind="ExternalInput")
    outw_d = nc.dram_tensor("outw", [C, C], F32, kind="ExternalInput")
    outb_d = nc.dram_tensor("outb", [128, C], F32, kind="ExternalInput")
    ggb_d = nc.dram_tensor("ggb", [128, 2 * 192], F32, kind="ExternalInput")
    gw_d = nc.dram_tensor("gw", [192, 3], F32, kind="ExternalInput")
    gbias_d = nc.dram_tensor("gbias", [128, 3], F32, kind="ExternalInput")
    fgb_d = nc.dram_tensor("fgb", [128, 2 * C], F32, kind="ExternalInput")
    fw1_d = nc.dram_tensor("fw1", [C, C], F32, kind="ExternalInput")
    fb1_d = nc.dram_tensor("fb1", [C, 1], F32, kind="ExternalInput")
    fw2_d = nc.dram_tensor("fw2", [C, C], F32, kind="ExternalInput")
    fb2_d = nc.dram_tensor("fb2", [128, C], F32, kind="ExternalInput")
    neglam_d = nc.dram_tensor("neglam", [C, 1], F32, kind="ExternalInput")
    out_d = nc.dram_tensor("out", [R, C], F32, kind="ExternalOutput")

    with tile.TileContext(nc) as tc:
        with (
            tc.tile_pool(name="wpool", bufs=1) as wp,
            tc.tile_pool(name="xpool", bufs=1) as xp,
            tc.tile_pool(name="upool", bufs=3) as up,
            tc.tile_pool(name="utpool", bufs=4) as utp,
            tc.tile_pool(name="zpool", bufs=1) as zp,
            tc.tile_pool(name="apool", bufs=1) as ap,
            tc.tile_pool(name="scratch", bufs=1) as sc1,
            tc.tile_pool(name="scratch2", bufs=2) as sc2,
            tc.tile_pool(name="g1ps", bufs=1, space="PSUM") as g1ps,
            tc.tile_pool(name="g2ps", bufs=1, space="PSUM") as g2ps,
            tc.tile_pool(name="auxps", bufs=1, space="PSUM") as auxps,
            tc.tile_pool(name="dram", bufs=1, space="DRAM") as dr,
        ):
            # ---- weights/constants (gpsimd queue; xt on sync for fast start) ----
            xt = xp.tile([128, 4 * R], F32R)
            for kk in range(4):
                nc.sync.dma_start(xt[:, kk * R:(kk + 1) * R], xt_d[kk * 128:(kk + 1) * 128, :])
            few1 = wp.tile([128, 4 * HID], F32R)
            for kk in range(4):
                nc.gpsimd.dma_start(few1[:, kk * HID:(kk + 1) * HID],
                                    few1_d[kk * 128:(kk + 1) * 128, :])
            few2 = wp.tile([HID, C], F32R)
            nc.gpsimd.dma_start(few2[:], few2_d[:])
            feb1 = wp.tile([HID, 1], F32)
            nc.gpsimd.dma_start(feb1[:], feb1_d[:])
            feb2 = wp.tile([C, 1], F32)
            nc.gpsimd.dma_start(feb2[:], feb2_d[:])
            maskt = wp.tile([128, RT], F32)
            nc.gpsimd.dma_start(maskt[:], mask_d[:])
            newe = wp.tile([128, RT], F32)
            nc.gpsimd.dma_start(newe[:], newe_d[:])
            wq = wp.tile([C, 2 * C], F32)
            nc.gpsimd.dma_start(wq[:], wq_d[:])
            wkv = wp.tile([C, 3 * C], F32)
            nc.gpsimd.dma_start(wkv[:], wkv_d[:])
            bq = wp.tile([C, 2], F32)
            nc.gpsimd.dma_start(bq[:], bq_d[:])
            bkv = wp.tile([128, 3 * C], F32)
            nc.gpsimd.dma_start(bkv[:], bkv_d[:])
            mhgb = wp.tile([128, 2 * C], F32)
            nc.gpsimd.dma_start(mhgb[:], mhgb_d[:])
            attgb = wp.tile([128, 2 * C], F32)
            nc.gpsimd.dma_start(attgb[:], attgb_d[:])
            outw = wp.tile([C, C], F32)
            nc.gpsimd.dma_start(outw[:], outw_d[:])
            outb = wp.tile([128, C], F32)
            nc.gpsimd.dma_start(outb[:], outb_d[:])
            ggb = wp.tile([128, 2 * 192], F32)
            nc.gpsimd.dma_start(ggb[:], ggb_d[:])
            gw1 = wp.tile([128, 3], F32)
            nc.gpsimd.dma_start(gw1[:], gw_d[0:128, :])
            gw2 = wp.tile([C, 3], F32)
            nc.gpsimd.dma_start(gw2[:], gw_d[128:192, :])
            gbias = wp.tile([128, 3], F32)
            nc.gpsimd.dma_start(gbias[:], gbias_d[:])
            fgb = wp.tile([128, 2 * C], F32)
            nc.gpsimd.dma_start(fgb[:], fgb_d[:])
            fw1 = wp.tile([C, C], F32)
            nc.gpsimd.dma_start(fw1[:], fw1_d[:])
            fb1 = wp.tile([C, 1], F32)
            nc.gpsimd.dma_start(fb1[:], fb1_d[:])
            fw2 = wp.tile([C, C], F32)
            nc.gpsimd.dma_start(fw2[:], fw2_d[:])
            fb2 = wp.tile([128, C], F32)
            nc.gpsimd.dma_start(fb2[:], fb2_d[:])
            neglam = wp.tile([C, 1], F32)
            nc.gpsimd.dma_start(neglam[:], neglam_d[:])

            ident = wp.tile([128, 128], F32)
            masks.make_identity(nc, ident[:])
            epst = wp.tile([128, 1], F32)
            nc.vector.memset(epst[:], EPS)

            # ---- stage A: feat encoder -> hT [64, R]; h token-major ----
            hT = ap.tile([C, R], F32)
            for c0, cw in TCH:
                p1 = auxps.tile([128, 512], F32, tag="aux", name=f"fe1_{c0}")
                for kk in range(4):
                    nc.tensor.matmul(p1[:, :cw], few1[:, kk * HID:(kk + 1) * HID],
                                     xt[:, kk * R + c0: kk * R + c0 + cw],
                                     start=(kk == 0), stop=(kk == 3))
                h1 = sc2.tile([128, 512], F32R, tag="h1", name=f"fe1r_{c0}")
                nc.scalar.activation(h1[:, :cw], p1[:, :cw], ACT.Relu, bias=feb1[:, 0:1])
                p2 = auxps.tile([128, 512], F32, tag="aux", name=f"fe2_{c0}")
                nc.tensor.matmul(p2[:C, :cw], few2[:], h1[:, :cw], start=True, stop=True)
                nc.scalar.activation(hT[:, c0:c0 + cw], p2[:C, :cw], ACT.Identity,
                                     bias=feb2[:, 0:1])

            h32 = ap.tile([128, RT * C], F32)
            hr = ap.tile([128, RT * C], F32R)
            for t in range(RT):
                pt = auxps.tile([128, 128], F32, tag="aux", name=f"htp_{t}")
                nc.tensor.matmul(pt[:, :C], hT[:, t * 128:(t + 1) * 128], ident[:C, :C],
                                 is_transpose=True)
                nc.scalar.activation(h32[:, t * C:(t + 1) * C], pt[:, :C], ACT.Copy,
                                     scale=maskt[:, t:t + 1])
                nc.scalar.activation(hr[:, t * C:(t + 1) * C], pt[:, :C], ACT.Copy,
                                     scale=maskt[:, t:t + 1])

            # ---- stage B: mh = LN(h); qT/k/v; k1v,k2v partials + kv AllReduce ----
            mh = ap.tile([128, RT * C], F32)
            mhm, mhinv = _ln_stats(nc, sc2, [h32[:]], C, epst, "mhln")
            nsc = sc2.tile([128, C], F32, tag="nsc", name="nsc_mh", bufs=2)
            for t in range(RT):
                _ln_norm(nc, h32[:, t * C:(t + 1) * C], mhm[:, t:t + 1], mhinv[:, t:t + 1],
                         mhgb[:, 0:C], mhgb[:, C:2 * C], mh[:, t * C:(t + 1) * C], nsc)

            mhT = ap.tile([C, R], F32)
            for t in range(RT):
                pt = auxps.tile([128, 128], F32, tag="aux", name=f"mhtp_{t}")
                nc.tensor.matmul(pt[:C, :], mh[:, t * C:(t + 1) * C], ident[:, :],
                                 is_transpose=True)
                nc.vector.tensor_copy(mhT[:, t * 128:(t + 1) * 128], pt[:C, :])

            q1T = ap.tile([C, R], F32)
            q2T = ap.tile([C, R], F32)
            for c0, cw in TCH:
                for qi, qT in enumerate([q1T, q2T]):
                    pq = auxps.tile([128, 512], F32, tag="aux", name=f"q{qi}_{c0}")
                    nc.tensor.matmul(pq[:C, :cw], wq[:, qi * C:(qi + 1) * C],
                                     mhT[:, c0:c0 + cw], start=True, stop=True)
                    nc.scalar.activation(qT[:, c0:c0 + cw], pq[:C, :cw], ACT.Identity,
                                         bias=bq[:, qi:qi + 1])

            kv = ap.tile([128, RT * 3 * C], F32)
            for t in range(RT):
                for wi in range(3):
                    pk = auxps.tile([128, 128], F32, tag="aux", name=f"kv{t}_{wi}")
                    nc.tensor.matmul(pk[:, :C], mhT[:, t * 128:(t + 1) * 128],
                                     wkv[:, wi * C:(wi + 1) * C], start=True, stop=True)
                    tkb = sc2.tile([128, C], F32, tag="kvb", name=f"kvb{t}_{wi}")
                    nc.vector.tensor_add(tkb[:], pk[:, :C], bkv[:, wi * C:(wi + 1) * C])
                    nc.scalar.activation(kv[:, (t * 3 + wi) * C:(t * 3 + wi + 1) * C],
                                         tkb[:], ACT.Copy, scale=maskt[:, t:t + 1])

            # k1v/k2v partial accumulate in a bank later reused by GEMM2
            pkv1 = g2ps.tile([128, 512], F32, tag="g2_0", name="pkv1")
            for t in range(RT):
                nc.tensor.matmul(pkv1[:C, 0:C], kv[:, (t * 3) * C:(t * 3 + 1) * C],
                                 kv[:, (t * 3 + 2) * C:(t * 3 + 3) * C],
                                 start=(t == 0), stop=(t == RT - 1))
            for t in range(RT):
                nc.tensor.matmul(pkv1[:C, C:2 * C], kv[:, (t * 3 + 1) * C:(t * 3 + 2) * C],
                                 kv[:, (t * 3 + 2) * C:(t * 3 + 3) * C],
                                 start=(t == 0), stop=(t == RT - 1))
            kvcat = sc1.tile([C, 2 * C], F32, tag="kvcat", name="kvcat")
            nc.vector.tensor_copy(kvcat[:], pkv1[:C, 0:2 * C])

            # ---- AG1: gather full h (f32r); per-core block is SBUF-native ----
            h_bi = dr.tile([128, RT * C], F32R, name="h_bi")
            h_bo = dr.tile([NCORES * 128, RT * C], F32R, name="h_bo", addr_space="Shared")
            nc.gpsimd.dma_start(h_bi[:], hr[:])
            nc.gpsimd.collective_compute(
                "AllGather", ALU.bypass, replica_groups=[list(range(NCORES))],
                ins=[h_bi[:]], outs=[h_bo[:]])
            hfull = zp.tile([128, PB * C], F32R, name="hfull")
            for ci in range(NCORES):
                nc.scalar.dma_start(hfull[:, ci * RT * C:(ci + 1) * RT * C],
                                    h_bo[ci * 128:(ci + 1) * 128, :])

            # ---- stage C: GEMM1 (a-sharded, contraction over all rows) ----
            putx = [g1ps.tile([C, cw], F32, tag=f"g1_{i}", name=f"g1_{i}")
                    for i, (c0, cw) in enumerate(TCH)]
            for nb in range(PB):
                uc = up.tile([128, R], F32R, tag="u", name=f"u_{nb}")
                nc.sync.dma_start(uc[:], u_d[nb * 128:(nb + 1) * 128, :])
                for i, (c0, cw) in enumerate(TCH):
                    nc.tensor.matmul(putx[i][:], hfull[:, nb * C:(nb + 1) * C],
                                     uc[:, c0:c0 + cw],
                                     start=(nb == 0), stop=(nb == PB - 1))
            utxT = ap.tile([C, R], F32)
            for i, (c0, cw) in enumerate(TCH):
                nc.scalar.copy(utxT[:, c0:c0 + cw], putx[i][:])

            # ---- stage D: z_loc = new_e_loc * utx_loc (token-major f32r) ----
            zloc = ap.tile([128, RT * C], F32R, name="zloc")
            for j in range(RT):
                pz = auxps.tile([128, 128], F32, tag="aux", name=f"zp_{j}")
                nc.tensor.matmul(pz[:, :C], utxT[:, j * 128:(j + 1) * 128],
                                 ident[:C, :C], is_transpose=True)
                nc.scalar.activation(zloc[:, j * C:(j + 1) * C], pz[:, :C],
                                     ACT.Copy, scale=newe[:, j:j + 1])

            # ---- AG2: gather z_loc + kv partials (block = [128, 640+128]) ----
            ZW = RT * C           # 640
            zkv_bi = dr.tile([128, ZW + 128], F32R, name="zkv_bi")
            zkv_bo = dr.tile([NCORES * 128, ZW + 128], F32R, name="zkv_bo",
                             addr_space="Shared")
            nc.gpsimd.dma_start(zkv_bi[:, 0:ZW], zloc[:])
            nc.gpsimd.dma_start(zkv_bi[0:C, ZW:ZW + 128], kvcat[:].bitcast(F32R))
            nc.gpsimd.dma_start(zkv_bi[C:128, ZW:ZW + 128], kvcat[:].bitcast(F32R))
            nc.gpsimd.collective_compute(
                "AllGather", ALU.bypass, replica_groups=[list(range(NCORES))],
                ins=[zkv_bi[:]], outs=[zkv_bo[:]])
            z = zp.tile([128, AB * C], F32R, name="z")
            kvg = sc1.tile([C, NCORES * 2 * C], F32, tag="kvg", name="kvg")
            for ci in range(NCORES):
                nc.scalar.dma_start(z[:, ci * ZW:(ci + 1) * ZW],
                                    zkv_bo[ci * 128:(ci + 1) * 128, 0:ZW])
                nc.gpsimd.dma_start(
                    kvg[:, ci * 2 * C:(ci + 1) * 2 * C].bitcast(F32R),
                    zkv_bo[ci * 128:ci * 128 + C, ZW:ZW + 128])
            kvred = wp.tile([C, 2 * C], F32)
            nc.vector.tensor_copy(kvred[:], kvg[:, 0:2 * C])
            for ci in range(1, NCORES):
                nc.vector.tensor_add(kvred[:], kvred[:], kvg[:, ci * 2 * C:(ci + 1) * 2 * C])
            k2vs = wp.tile([C, C], F32)
            nc.scalar.activation(k2vs[:], kvred[:, C:2 * C], ACT.Copy,
                                 scale=neglam[:, 0:1])

            # ---- stage F: xa = q1@k1v - lam*q2@k2v; LN; out proj (early) ----
            xa_all = ap.tile([128, RT * C], F32)
            for t in range(RT):
                pxa = auxps.tile([128, 128], F32, tag="aux", name=f"xa_{t}")
                nc.tensor.matmul(pxa[:, :C], q1T[:, t * 128:(t + 1) * 128],
                                 kvred[:, 0:C], start=True, stop=False)
                nc.tensor.matmul(pxa[:, :C], q2T[:, t * 128:(t + 1) * 128],
                                 k2vs[:], start=False, stop=True)
                nc.vector.tensor_copy(xa_all[:, t * C:(t + 1) * C], pxa[:, :C])
            xam, xainv = _ln_stats(nc, sc2, [xa_all[:]], C, epst, "xaln")
            xaLNT = ap.tile([C, R], F32)
            nsc2 = sc2.tile([128, C], F32, tag="nsc", name="nsc_xa", bufs=2)
            for t in range(RT):
                xaln = sc2.tile([128, C], F32, tag="xaln", name=f"xaln{t}")
                _ln_norm(nc, xa_all[:, t * C:(t + 1) * C], xam[:, t:t + 1],
                         xainv[:, t:t + 1], attgb[:, 0:C], attgb[:, C:2 * C],
                         xaln, nsc2)
                pxt = auxps.tile([128, 128], F32, tag="aux", name=f"xat_{t}")
                nc.tensor.matmul(pxt[:C, :], xaln[:], ident[:, :], is_transpose=True)
                nc.vector.tensor_copy(xaLNT[:, t * 128:(t + 1) * 128], pxt[:C, :])
            mha = ap.tile([128, RT * C], F32)
            for t in range(RT):
                pm = auxps.tile([128, 128], F32, tag="aux", name=f"mha_{t}")
                nc.tensor.matmul(pm[:, :C], xaLNT[:, t * 128:(t + 1) * 128],
                                 outw[:], start=True, stop=True)
                nc.vector.tensor_add(mha[:, t * C:(t + 1) * C], pm[:, :C], outb[:])

            # ---- stage E: GEMM2 ----
            pf = [g2ps.tile([C, cw], F32, tag=f"g2_{i}", name=f"g2_{i}")
                  for i, (c0, cw) in enumerate(TCH)]
            for ab in range(AB):
                utt = utp.tile([128, R], F32R, tag="ut", name=f"ut_{ab}")
                nc.scalar.dma_start(utt[:], ut_d[ab * 128:(ab + 1) * 128, :])
                for i, (c0, cw) in enumerate(TCH):
                    nc.tensor.matmul(pf[i][:], z[:, ab * C:(ab + 1) * C],
                                     utt[:, c0:c0 + cw],
                                     start=(ab == 0), stop=(ab == AB - 1))
            furT = ap.tile([C, R], F32)
            for i, (c0, cw) in enumerate(TCH):
                nc.scalar.copy(furT[:, c0:c0 + cw], pf[i][:])
            fur = ap.tile([128, RT * C], F32)
            for t in range(RT):
                ptf = g1ps.tile([128, 128], F32, tag=f"g1_{t % NSUB}", name=f"ftp_{t}")
                nc.tensor.matmul(ptf[:, :C], furT[:, t * 128:(t + 1) * 128],
                                 ident[:C, :C], is_transpose=True)
                nc.vector.tensor_copy(fur[:, t * C:(t + 1) * C], ptf[:, :C])

            # ---- stage G: gate LN (over h|mha|fur), softmax, mix ----
            gm, ginv = _ln_stats(nc, sc2, [h32[:], mha[:], fur[:]], 192, epst, "gln")
            lg_all = sc1.tile([128, RT * 3], F32, tag="lg", name="lg_all")
            catln = sc2.tile([128, 192], F32, tag="catln", name="catln", bufs=2)
            nsc3 = sc2.tile([128, C], F32, tag="nsc", name="nsc_g", bufs=2)
            for t in range(RT):
                _ln_norm(nc, h32[:, t * C:(t + 1) * C], gm[:, t:t + 1], ginv[:, t:t + 1],
                         ggb[:, 0:C], ggb[:, 192:192 + C], catln[:, 0:C], nsc3)
                _ln_norm(nc, mha[:, t * C:(t + 1) * C], gm[:, t:t + 1], ginv[:, t:t + 1],
                         ggb[:, C:2 * C], ggb[:, 192 + C:192 + 2 * C],
                         catln[:, C:2 * C], nsc3)
                _ln_norm(nc, fur[:, t * C:(t + 1) * C], gm[:, t:t + 1], ginv[:, t:t + 1],
                         ggb[:, 2 * C:192], ggb[:, 192 + 2 * C:2 * 192],
                         catln[:, 2 * C:192], nsc3)
                pc1 = g1ps.tile([128, 128], F32, tag="g1_0", name=f"ct1_{t}")
                nc.tensor.matmul(pc1[:, :], catln[:, 0:128], ident[:, :], is_transpose=True)
                ct1 = sc2.tile([128, 128], F32, tag="ct1", name=f"ct1s{t}")
                nc.vector.tensor_copy(ct1[:], pc1[:, :])
                pc2 = g1ps.tile([128, 128], F32, tag="g1_1", name=f"ct2_{t}")
                nc.tensor.matmul(pc2[:C, :], catln[:, 128:192], ident[:, :], is_transpose=True)
                ct2 = sc2.tile([C, 128], F32, tag="ct2", name=f"ct2s{t}")
                nc.vector.tensor_copy(ct2[:], pc2[:C, :])
                pl = g1ps.tile([128, 128], F32, tag="g1_2", name=f"lg_{t}")
                nc.tensor.matmul(pl[:, 0:3], ct1[:], gw1[:], start=True, stop=False)
                nc.tensor.matmul(pl[:, 0:3], ct2[:], gw2[:], start=False, stop=True)
                nc.vector.tensor_add(lg_all[:, t * 3:(t + 1) * 3], pl[:, 0:3], gbias[:])
            # batched softmax over [128, RT, 3]
            lgv = lg_all[:].rearrange("p (t c) -> p t c", t=RT)
            lmax = sc2.tile([128, RT], F32, tag="lmax", name="lmax")
            nc.vector.reduce_max(lmax[:], lgv, axis=AX.X)
            for t in range(RT):
                nc.vector.tensor_scalar(lg_all[:, t * 3:(t + 1) * 3],
                                        lg_all[:, t * 3:(t + 1) * 3],
                                        lmax[:, t:t + 1], None, op0=ALU.subtract)
            nc.scalar.activation(lg_all[:], lg_all[:], ACT.Exp)
            lsum = sc2.tile([128, RT], F32, tag="lsum", name="lsum")
            nc.vector.reduce_sum(lsum[:], lgv, axis=AX.X)
            linv = sc2.tile([128, RT], F32, tag="linv", name="linv")
            nc.vector.reciprocal(linv[:], lsum[:])
            for t in range(RT):
                nc.vector.tensor_scalar(lg_all[:, t * 3:(t + 1) * 3],
                                        lg_all[:, t * 3:(t + 1) * 3],
                                        linv[:, t:t + 1], None, op0=ALU.mult)

            mixs = ap.tile([128, RT * C], F32)
            for t in range(RT):
                mix = mixs[:, t * C:(t + 1) * C]
                mt = sc2.tile([128, C], F32, tag="mixt", name=f"mixt{t}")
                nc.vector.tensor_scalar(mix[:], h32[:, t * C:(t + 1) * C],
                                        lg_all[:, t * 3:t * 3 + 1], None, op0=ALU.mult)
                nc.vector.tensor_scalar(mt[:], mha[:, t * C:(t + 1) * C],
                                        lg_all[:, t * 3 + 1:t * 3 + 2], None, op0=ALU.mult)
                nc.vector.tensor_add(mix[:], mix[:], mt[:])
                nc.vector.tensor_scalar(mt[:], fur[:, t * C:(t + 1) * C],
                                        lg_all[:, t * 3 + 2:t * 3 + 3], None, op0=ALU.mult)
                nc.vector.tensor_add(mix[:], mix[:], mt[:])

            # ---- stage H: FFN + residual -> out ----
            fm, finv = _ln_stats(nc, sc2, [mixs[:]], C, epst, "ffnln")
            fT = ap.tile([C, R], F32)
            nsc4 = sc2.tile([128, C], F32, tag="nsc", name="nsc_f", bufs=2)
            for t in range(RT):
                fln = sc2.tile([128, C], F32, tag="fln", name=f"fln{t}")
                _ln_norm(nc, mixs[:, t * C:(t + 1) * C], fm[:, t:t + 1], finv[:, t:t + 1],
                         fgb[:, 0:C], fgb[:, C:2 * C], fln, nsc4)
                pft = g1ps.tile([128, 128], F32, tag="g1_3", name=f"fT_{t}")
                nc.tensor.matmul(pft[:C, :], fln[:], ident[:, :], is_transpose=True)
                nc.vector.tensor_copy(fT[:, t * 128:(t + 1) * 128], pft[:C, :])
            g1T = ap.tile([C, R], F32)
            for ci, (c0, cw) in enumerate(TCH):
                pg = g1ps.tile([128, 512], F32, tag=f"g1_{ci}", name=f"ffn1_{c0}")
                nc.tensor.matmul(pg[:C, :cw], fw1[:], fT[:, c0:c0 + cw],
                                 start=True, stop=True)
                nc.scalar.activation(g1T[:, c0:c0 + cw], pg[:C, :cw], ACT.Gelu,
                                     bias=fb1[:, 0:1])
            for t in range(RT):
                pf2 = g1ps.tile([128, 128], F32, tag=f"g1_{t % NSUB}", name=f"ffn2_{t}")
                nc.tensor.matmul(pf2[:, :C], g1T[:, t * 128:(t + 1) * 128], fw2[:],
                                 start=True, stop=True)
                ot = sc2.tile([128, C], F32, tag="ot", name=f"ot{t}")
                nc.vector.tensor_add(ot[:], pf2[:, :C], fb2[:])
                nc.vector.tensor_add(ot[:], ot[:], mixs[:, t * C:(t + 1) * C])
                nc.gpsimd.dma_start(out_d[t * 128:(t + 1) * 128, :], ot[:])

    nc.compile()
    return nc


def _host_new_e(e, freq_deltas, freq_bias, readout_w, readout_b, alpha_w):
    e = e.astype(np.float64)
    deltas = np.log1p(np.exp(freq_deltas.astype(np.float64))) + DMIN
    freqs = np.cumsum(deltas) + float(freq_bias)
    freqs = OMEGA * np.tanh(freqs / OMEGA)
    powers = e[:, None] ** np.arange(1, K + 1)
    phase = powers[:, :, None] * freqs
    ns = NF ** 0.5
    rw = readout_w.astype(np.float64)
    eig = (rw[:, 0][None, :]
           + np.einsum('nkf,kf->nk', np.sin(phase) / ns, rw[:, 1:1 + NF])
           + np.einsum('nkf,kf->nk', np.cos(phase) / ns, rw[:, 1 + NF:])
           + readout_b.astype(np.float64)[None, :])
    new_e = eig @ alpha_w.astype(np.float64)
    return new_e[:, 0].astype(np.float32)


def kernel(**inp):
    global LAST_RESULT
    import time as _time
    _t0 = _time.time()
    if "nc" not in _CACHE:
        _CACHE["nc"] = build_nc()
    nc = _CACHE["nc"]
    _t1 = _time.time()

    f32 = np.float32
    e = np.asarray(inp["e"], f32)
    u = np.asarray(inp["u"], f32)
    x = np.asarray(inp["x"], f32)

    new_e = _host_new_e(e, np.asarray(inp["freq_deltas"]), np.asarray(inp["freq_bias"]),
                        np.asarray(inp["readout_w"]), np.asarray(inp["readout_b"]),
                        np.asarray(inp["alpha_w"]))
    new_e_pad = np.zeros(P, f32)
    new_e_pad[:N] = new_e
    newe_t = np.ascontiguousarray(new_e_pad.reshape(AB, 128).T)

    lam1 = float(np.exp(np.sum(np.asarray(inp["lq1"], f32) * np.asarray(inp["lk1"], f32))))
    lam2 = float(np.exp(np.sum(np.asarray(inp["lq2"], f32) * np.asarray(inp["lk2"], f32))))
    lam_full = lam1 - lam2 + LAM_INIT

    def bc(v, n=128):
        return np.ascontiguousarray(np.tile(np.asarray(v, f32)[None, :], (n, 1)))

    def col(v):
        return np.ascontiguousarray(np.asarray(v, f32)[:, None])

    u_pad = np.zeros((P, P), f32)
    u_pad[:N, :N] = u
    x_pad = np.zeros((P, NFEAT), f32)
    x_pad[:N] = x

    common = {
        "few1": np.ascontiguousarray(inp["fe_w1"], dtype=f32),
        "few2": np.ascontiguousarray(inp["fe_w2"], dtype=f32),
        "feb1": col(inp["fe_b1"]),
        "feb2": col(inp["fe_b2"]),
        "wq": np.ascontiguousarray(np.concatenate([inp["wq1"], inp["wq2"]], axis=1), dtype=f32),
        "wkv": np.ascontiguousarray(np.concatenate([inp["wk1"], inp["wk2"], inp["wv"]], axis=1), dtype=f32),
        "bq": np.ascontiguousarray(np.stack([inp["bq1"], inp["bq2"]], axis=1), dtype=f32),
        "bkv": bc(np.concatenate([inp["bk1"], inp["bk2"], inp["bv"]])),
        "mhgb": bc(np.concatenate([inp["mha_ln_g"], inp["mha_ln_b"]])),
        "attgb": bc(np.concatenate([(1.0 - LAM_INIT) * np.asarray(inp["attn_ln_g"], f32),
                                    (1.0 - LAM_INIT) * np.asarray(inp["attn_ln_b"], f32)])),
        "outw": np.ascontiguousarray(inp["out_w"], dtype=f32),
        "outb": bc(inp["out_b"]),
        "ggb": bc(np.concatenate([inp["gate_ln_g"], inp["gate_ln_b"]])),
        "gw": np.ascontiguousarray(inp["gate_w"], dtype=f32),
        "gbias": bc(inp["gate_b"]),
        "fgb": bc(np.concatenate([inp["ffn_ln_g"], inp["ffn_ln_b"]])),
        "fw1": np.ascontiguousarray(inp["ffn_w1"], dtype=f32),
        "fb1": col(inp["ffn_b1"]),
        "fw2": np.ascontiguousarray(inp["ffn_w2"], dtype=f32),
        "fb2": bc(inp["ffn_b2"]),
        "neglam": np.full((C, 1), -lam_full, f32),
    }

    in_maps = []
    for ci in range(NCORES):
        r0, r1 = ci * R, (ci + 1) * R
        mask = np.zeros((128, RT), f32)
        for t in range(RT):
            base = r0 + t * 128
            nreal = min(max(N - base, 0), 128)
            mask[:nreal, t] = 1.0
        m = dict(common)
        m["u"] = np.ascontiguousarray(u_pad[:, r0:r1])
        m["ut"] = np.ascontiguousarray(u_pad[r0:r1].T)
        m["xt"] = np.ascontiguousarray(x_pad[r0:r1].T)
        m["mask"] = mask
        m["newe"] = np.ascontiguousarray(newe_t[:, ci * RT:(ci + 1) * RT])
        in_maps.append(m)

    _t2 = _time.time()
    res = run_bass_kernel_spmd(nc, in_maps, list(range(NCORES)))
    _t3 = _time.time()
    print(f"[kernel] build+compile={_t1-_t0:.1f}s hostprep={_t2-_t1:.1f}s run={_t3-_t2:.1f}s")
    LAST_RESULT = res
    out = np.concatenate([res.results[ci]["out"] for ci in range(NCORES)], axis=0)
    return out[:N]


# revision 23
# speedup vs baseline: 1.0318x; 1.0318x over previous
"""NoFoDifformer Trainium2 kernel — 8-core SPMD.

Sharding (v3): 1280 rows of x/h per core; u column-sharded for GEMM1 and
row-sharded(transposed) for GEMM2; exactly TWO collectives, both AllGathers:
  AG1: h (each core's feat-encoder rows -> full h, f32r) before GEMM1.
  GEMM1  utx[:, cols_i] = u[:, cols_i]^T h : exact per-core a-slice; lhsT =
         h-full tiles (f32r), moving = u_col row-band chunks -> utx^T local.
  z_loc = new_e_loc * utx_loc (token-major f32r).
  AG2: z_loc + local k1v/k2v partials -> full z + all partials (summed
       locally). No AllReduce chain: collectives are the expensive barrier.
  GEMM2  h_fur rows_i = u[rows_i,:] (z): lhsT = z (f32r), moving = uT chunks.
new_e (eig encoding) is computed on host (tiny).
Both big GEMMs run in float32r (PE full-rate fp32, ~1.5e-4 rel); all small
GEMMs plain fp32. LayerNorm/softmax statistics are batched across the 10
token tiles ([128, 10] stat rows) to keep DVE/ACT instruction counts low.
"""
import numpy as np

import concourse.bass as bass
import concourse.tile as tile
from concourse import bacc, mybir, masks
from concourse.bass_utils import run_bass_kernel_spmd

F32 = mybir.dt.float32
F32R = mybir.dt.float32r
AX = mybir.AxisListType
ALU = mybir.AluOpType
ACT = mybir.ActivationFunctionType

NCORES = 8
N = 10000
P = 10240            # padded
R = P // NCORES      # 1280 rows per core
RT = R // 128        # 10 token tiles per core
C = 64
NFEAT = 512
HID = 128
K = 10
NF = 16
OMEGA = 50.0
DMIN = 0.25
EPS = 1e-5
LAM_INIT = 0.8 - 0.6 * float(np.exp(-0.3 * 0))   # layer 0 -> 0.2

AB = P // 128        # 80 a-blocks (z)
PB = P // 128        # 80 n row-blocks of full h
NSUB = 4             # tail psum tag rotation
TCH = [(0, 512), (512, 512), (1024, 256)]   # token chunks of the 1280 rows

_CACHE = {}
LAST_RESULT = None


def _ln_stats(nc, sp, pieces, d, epst, name):
    """Batched LN stats over RT token tiles. pieces: APs [128, RT*ci] whose
    free dim is (t, ci). Returns (m, inv) each [128, RT]."""
    s = sp.tile([128, RT], F32, tag="ln_s", name=f"{name}_s")
    q = sp.tile([128, RT], F32, tag="ln_q", name=f"{name}_q")
    tmp = sp.tile([128, RT], F32, tag="ln_tmp", name=f"{name}_tmp")
    sq = sp.tile([128, RT * C], F32, tag="ln_sq", name=f"{name}_sq", bufs=1)
    for i, x in enumerate(pieces):
        ci = x.shape[1] // RT
        xv = x.rearrange("p (t c) -> p t c", t=RT)
        nc.vector.reduce_sum((s if i == 0 else tmp)[:], xv, axis=AX.X)
        if i > 0:
            nc.vector.tensor_add(s[:], s[:], tmp[:])
        nc.vector.tensor_tensor(sq[:, : RT * ci], x[:], x[:], op=ALU.mult)
        sqv = sq[:, : RT * ci].rearrange("p (t c) -> p t c", t=RT)
        nc.vector.reduce_sum((q if i == 0 else tmp)[:], sqv, axis=AX.X)
        if i > 0:
            nc.vector.tensor_add(q[:], q[:], tmp[:])
    m = sp.tile([128, RT], F32, tag="ln_m", name=f"{name}_m")
    nc.vector.tensor_scalar_mul(m[:], s[:], 1.0 / d)
    nc.vector.tensor_scalar_mul(q[:], q[:], 1.0 / d)
    nc.vector.tensor_tensor(tmp[:], m[:], m[:], op=ALU.mult)
    nc.vector.tensor_sub(q[:], q[:], tmp[:])
    std = sp.tile([128, RT], F32, tag="ln_std", name=f"{name}_std")
    nc.scalar.activation(std[:], q[:], ACT.Sqrt, bias=epst[:, 0:1])
    inv = sp.tile([128, RT], F32, tag="ln_inv", name=f"{name}_inv")
    nc.vector.reciprocal(inv[:], std[:])
    return m, inv


def _ln_norm(nc, x, m1, inv1, g, b, out, scratch):
    """out = (x - m1)*inv1*g + b for one [128, ci] tile (m1/inv1: [128,1])."""
    nc.vector.tensor_scalar(scratch[:], x[:], m1, inv1,
                            op0=ALU.subtract, op1=ALU.mult)
    nc.vector.tensor_tensor(scratch[:], scratch[:], g[:], op=ALU.mult)
    nc.vector.tensor_tensor(out[:], scratch[:], b[:], op=ALU.add)


def build_nc():
    nc = bacc.Bacc("TRN2", target_bir_lowering=False, debug=False, num_devices=NCORES)

    # ---- I/O ----
    u_d = nc.dram_tensor("u", [P, R], F32R, kind="ExternalInput")   # u[:, cols_i]
    ut_d = nc.dram_tensor("ut", [P, R], F32R, kind="ExternalInput")
    xt_d = nc.dram_tensor("xt", [NFEAT, R], F32R, kind="ExternalInput")
    pr_d = nc.dram_tensor("packr", [128, 4 * HID + C], F32R, kind="ExternalInput")
    p128_d = nc.dram_tensor("pack128", [128, 1120], F32, kind="ExternalInput")
    p64_d = nc.dram_tensor("pack64", [C, 712], F32, kind="ExternalInput")
    out_d = nc.dram_tensor("out", [R, C], F32, kind="ExternalOutput")

    with tile.TileContext(nc) as tc:
        with (
            tc.tile_pool(name="wpool", bufs=1) as wp,
            tc.tile_pool(name="xpool", bufs=1) as xp,
            tc.tile_pool(name="upool", bufs=3) as up,
            tc.tile_pool(name="utpool", bufs=8) as utp,
            tc.tile_pool(name="apool", bufs=1) as ap,
            tc.tile_pool(name="scratch", bufs=1) as sc1,
            tc.tile_pool(name="scratch2", bufs=2) as sc2,
            tc.tile_pool(name="g1ps", bufs=1, space="PSUM") as g1ps,
            tc.tile_pool(name="g2ps", bufs=1, space="PSUM") as g2ps,
            tc.tile_pool(name="auxps", bufs=1, space="PSUM") as auxps,
            tc.tile_pool(name="dram", bufs=1, space="DRAM") as dr,
        ):
            # ---- packed weights/constants: 3 DMAs ----
            xt = xp.tile([128, 4 * R], F32R)
            for kk in range(4):
                nc.sync.dma_start(xt[:, kk * R:(kk + 1) * R], xt_d[kk * 128:(kk + 1) * 128, :])
            packr = wp.tile([128, 4 * HID + C], F32R)
            nc.gpsimd.dma_start(packr[:], pr_d[:])
            p128 = wp.tile([128, 1120], F32)
            nc.gpsimd.dma_start(p128[:], p128_d[:])
            p64 = wp.tile([C, 712], F32)
            nc.gpsimd.dma_start(p64[:], p64_d[:])

            few1 = packr[:, 0:4 * HID]
            few2 = packr[0:HID, 4 * HID:4 * HID + C]
            # p128 cols: mhgb 128 | attgb 128 | outb 64 | ggb 384 | gbias 3 | fgb 128
            #            | bkv 192 | maskt 10 | newe 10 | feb1 1 | gw1 3
            mhgb = p128[:, 0:128]
            attgb = p128[:, 128:256]
            outb = p128[:, 256:320]
            ggb = p128[:, 320:704]
            gbias = p128[:, 704:707]
            fgb = p128[:, 707:835]
            bkv = p128[:, 835:1027]
            maskt = p128[:, 1027:1037]
            newe = p128[:, 1037:1047]
            feb1 = p128[:, 1047:1048]
            gw1 = p128[:, 1048:1051]
            fb2 = p128[:, 1051:1115]
            # p64 cols: wq 128 | wkv 192 | bq 2 | outw 64 | fw1 64 | fb1 1 | fw2 64
            #           | feb2 1 | neglam 1 | gw2 3 | (pad)
            wq = p64[:, 0:128]
            wkv = p64[:, 128:320]
            bq = p64[:, 320:322]
            outw = p64[:, 322:386]
            fw1 = p64[:, 386:450]
            fb1 = p64[:, 450:451]
            fw2 = p64[:, 451:515]
            feb2 = p64[:, 515:516]
            neglam = p64[:, 516:517]
            gw2 = p64[:, 517:520]

            ident = wp.tile([128, 128], F32)
            masks.make_identity(nc, ident[:])
            epst = wp.tile([128, 1], F32)
            nc.vector.memset(epst[:], EPS)

            # ---- stage A: feat encoder -> hT [64, R]; h token-major ----
            hT = ap.tile([C, R], F32)
            for c0, cw in TCH:
                p1 = auxps.tile([128, 512], F32, tag="aux", name=f"fe1_{c0}")
                for kk in range(4):
                    nc.tensor.matmul(p1[:, :cw], few1[:, kk * HID:(kk + 1) * HID],
                                     xt[:, kk * R + c0: kk * R + c0 + cw],
                                     start=(kk == 0), stop=(kk == 3))
                h1 = sc2.tile([128, 512], F32R, tag="h1", name=f"fe1r_{c0}")
                nc.scalar.activation(h1[:, :cw], p1[:, :cw], ACT.Relu, bias=feb1[:, 0:1])
                p2 = auxps.tile([128, 512], F32, tag="aux", name=f"fe2_{c0}")
                nc.tensor.matmul(p2[:C, :cw], few2[:], h1[:, :cw], start=True, stop=True)
                nc.scalar.activation(hT[:, c0:c0 + cw], p2[:C, :cw], ACT.Identity,
                                     bias=feb2[:, 0:1])

            h32 = ap.tile([128, RT * C], F32)
            hr = ap.tile([128, RT * C], F32R)
            for t in range(RT):
                pt = auxps.tile([128, 128], F32, tag="aux", name=f"htp_{t}")
                nc.tensor.matmul(pt[:, :C], hT[:, t * 128:(t + 1) * 128], ident[:C, :C],
                                 is_transpose=True)
                nc.scalar.activation(h32[:, t * C:(t + 1) * C], pt[:, :C], ACT.Copy,
                                     scale=maskt[:, t:t + 1])
                nc.scalar.activation(hr[:, t * C:(t + 1) * C], pt[:, :C], ACT.Copy,
                                     scale=maskt[:, t:t + 1])

            # ---- stage B: mh = LN(h); qT/k/v; k1v,k2v partials + kv AllReduce ----
            mh = ap.tile([128, RT * C], F32)
            mhm, mhinv = _ln_stats(nc, sc2, [h32[:]], C, epst, "mhln")
            nsc = sc2.tile([128, C], F32, tag="nsc", name="nsc_mh", bufs=2)
            for t in range(RT):
                _ln_norm(nc, h32[:, t * C:(t + 1) * C], mhm[:, t:t + 1], mhinv[:, t:t + 1],
                         mhgb[:, 0:C], mhgb[:, C:2 * C], mh[:, t * C:(t + 1) * C], nsc)

            mhT = ap.tile([C, R], F32)
            for t in range(RT):
                pt = auxps.tile([128, 128], F32, tag="aux", name=f"mhtp_{t}")
                nc.tensor.matmul(pt[:C, :], mh[:, t * C:(t + 1) * C], ident[:, :],
                                 is_transpose=True)
                nc.vector.tensor_copy(mhT[:, t * 128:(t + 1) * 128], pt[:C, :])

            q1T = ap.tile([C, R], F32)
            q2T = ap.tile([C, R], F32)
            for c0, cw in TCH:
                for qi, qT in enumerate([q1T, q2T]):
                    pq = auxps.tile([128, 512], F32, tag="aux", name=f"q{qi}_{c0}")
                    nc.tensor.matmul(pq[:C, :cw], wq[:, qi * C:(qi + 1) * C],
                                     mhT[:, c0:c0 + cw], start=True, stop=True)
                    nc.scalar.activation(qT[:, c0:c0 + cw], pq[:C, :cw], ACT.Identity,
                                         bias=bq[:, qi:qi + 1])

            kv = ap.tile([128, RT * 3 * C], F32)
            for t in range(RT):
                for wi in range(3):
                    pk = auxps.tile([128, 128], F32, tag="aux", name=f"kv{t}_{wi}")
                    nc.tensor.matmul(pk[:, :C], mhT[:, t * 128:(t + 1) * 128],
                                     wkv[:, wi * C:(wi + 1) * C], start=True, stop=True)
                    tkb = sc2.tile([128, C], F32, tag="kvb", name=f"kvb{t}_{wi}")
                    nc.vector.tensor_add(tkb[:], pk[:, :C], bkv[:, wi * C:(wi + 1) * C])
                    nc.scalar.activation(kv[:, (t * 3 + wi) * C:(t * 3 + wi + 1) * C],
                                         tkb[:], ACT.Copy, scale=maskt[:, t:t + 1])

            # k1v/k2v partial accumulate in a bank later reused by GEMM2
            pkv1 = g2ps.tile([128, 512], F32, tag="g2_0", name="pkv1")
            for t in range(RT):
                nc.tensor.matmul(pkv1[:C, 0:C], kv[:, (t * 3) * C:(t * 3 + 1) * C],
                                 kv[:, (t * 3 + 2) * C:(t * 3 + 3) * C],
                                 start=(t == 0), stop=(t == RT - 1))
            for t in range(RT):
                nc.tensor.matmul(pkv1[:C, C:2 * C], kv[:, (t * 3 + 1) * C:(t * 3 + 2) * C],
                                 kv[:, (t * 3 + 2) * C:(t * 3 + 3) * C],
                                 start=(t == 0), stop=(t == RT - 1))
            kvcat = sc1.tile([C, 2 * C], F32, tag="kvcat", name="kvcat")
            nc.vector.tensor_copy(kvcat[:], pkv1[:C, 0:2 * C])

            # ---- AG1: gather full h (f32r); per-core block is SBUF-native ----
            h_bi = dr.tile([128, RT * C], F32R, name="h_bi")
            h_bo = dr.tile([NCORES * 128, RT * C], F32R, name="h_bo", addr_space="Shared")
            nc.gpsimd.dma_start(h_bi[:], hr[:])
            nc.gpsimd.collective_compute(
                "AllGather", ALU.bypass, replica_groups=[list(range(NCORES))],
                ins=[h_bi[:]], outs=[h_bo[:]])


            # ---- stage C: GEMM1 (a-sharded, contraction over all rows) ----
            putx = [g1ps.tile([C, cw], F32, tag=f"g1_{i}", name=f"g1_{i}")
                    for i, (c0, cw) in enumerate(TCH)]
            for nb in range(PB):
                hf = utp.tile([128, C], F32R, tag="hf", name=f"hf_{nb}", bufs=6)
                ci_, t_ = divmod(nb, RT)
                nc.scalar.dma_start(hf[:], h_bo[ci_ * 128:(ci_ + 1) * 128, t_ * C:(t_ + 1) * C])
                uc = up.tile([128, R], F32R, tag="u", name=f"u_{nb}")
                nc.sync.dma_start(uc[:], u_d[nb * 128:(nb + 1) * 128, :])
                for i, (c0, cw) in enumerate(TCH):
                    nc.tensor.matmul(putx[i][:], hf[:],
                                     uc[:, c0:c0 + cw],
                                     start=(nb == 0), stop=(nb == PB - 1))
            utxT = ap.tile([C, R], F32)
            for i, (c0, cw) in enumerate(TCH):
                nc.scalar.copy(utxT[:, c0:c0 + cw], putx[i][:])

            # ---- stage D: z_loc = new_e_loc * utx_loc (token-major f32r) ----
            zloc = ap.tile([128, RT * C], F32R, name="zloc")
            for j in range(RT):
                pz = auxps.tile([128, 128], F32, tag="aux", name=f"zp_{j}")
                nc.tensor.matmul(pz[:, :C], utxT[:, j * 128:(j + 1) * 128],
                                 ident[:C, :C], is_transpose=True)
                nc.scalar.activation(zloc[:, j * C:(j + 1) * C], pz[:, :C],
                                     ACT.Copy, scale=newe[:, j:j + 1])

            # ---- AG2: gather z_loc + kv partials (block = [128, 640+128]) ----
            ZW = RT * C           # 640
            zkv_bi = dr.tile([128, ZW + 128], F32R, name="zkv_bi")
            zkv_bo = dr.tile([NCORES * 128, ZW + 128], F32R, name="zkv_bo",
                             addr_space="Shared")
            nc.gpsimd.dma_start(zkv_bi[:, 0:ZW], zloc[:])
            nc.gpsimd.dma_start(zkv_bi[0:C, ZW:ZW + 128], kvcat[:].bitcast(F32R))
            nc.gpsimd.dma_start(zkv_bi[C:128, ZW:ZW + 128], kvcat[:].bitcast(F32R))
            nc.gpsimd.collective_compute(
                "AllGather", ALU.bypass, replica_groups=[list(range(NCORES))],
                ins=[zkv_bi[:]], outs=[zkv_bo[:]])
            kvg = sc1.tile([C, NCORES * 2 * C], F32, tag="kvg", name="kvg")
            for ci in range(NCORES):
                nc.gpsimd.dma_start(
                    kvg[:, ci * 2 * C:(ci + 1) * 2 * C].bitcast(F32R),
                    zkv_bo[ci * 128:ci * 128 + C, ZW:ZW + 128])
            kvred = wp.tile([C, 2 * C], F32)
            nc.vector.tensor_copy(kvred[:], kvg[:, 0:2 * C])
            for ci in range(1, NCORES):
                nc.vector.tensor_add(kvred[:], kvred[:], kvg[:, ci * 2 * C:(ci + 1) * 2 * C])
            k2vs = wp.tile([C, C], F32)
            nc.scalar.activation(k2vs[:], kvred[:, C:2 * C], ACT.Copy,
                                 scale=neglam[:, 0:1])

            # ---- stage F: xa = q1@k1v - lam*q2@k2v; LN; out proj (early) ----
            xa_all = ap.tile([128, RT * C], F32)
            for t in range(RT):
                pxa = auxps.tile([128, 128], F32, tag="aux", name=f"xa_{t}")
                nc.tensor.matmul(pxa[:, :C], q1T[:, t * 128:(t + 1) * 128],
                                 kvred[:, 0:C], start=True, stop=False)
                nc.tensor.matmul(pxa[:, :C], q2T[:, t * 128:(t + 1) * 128],
                                 k2vs[:], start=False, stop=True)
                nc.vector.tensor_copy(xa_all[:, t * C:(t + 1) * C], pxa[:, :C])
            xam, xainv = _ln_stats(nc, sc2, [xa_all[:]], C, epst, "xaln")
            xaLNT = ap.tile([C, R], F32)
            nsc2 = sc2.tile([128, C], F32, tag="nsc", name="nsc_xa", bufs=2)
            for t in range(RT):
                xaln = sc2.tile([128, C], F32, tag="xaln", name=f"xaln{t}")
                _ln_norm(nc, xa_all[:, t * C:(t + 1) * C], xam[:, t:t + 1],
                         xainv[:, t:t + 1], attgb[:, 0:C], attgb[:, C:2 * C],
                         xaln, nsc2)
                pxt = auxps.tile([128, 128], F32, tag="aux", name=f"xat_{t}")
                nc.tensor.matmul(pxt[:C, :], xaln[:], ident[:, :], is_transpose=True)
                nc.vector.tensor_copy(xaLNT[:, t * 128:(t + 1) * 128], pxt[:C, :])
            mha = ap.tile([128, RT * C], F32)
            for t in range(RT):
                pm = auxps.tile([128, 128], F32, tag="aux", name=f"mha_{t}")
                nc.tensor.matmul(pm[:, :C], xaLNT[:, t * 128:(t + 1) * 128],
                                 outw[:], start=True, stop=True)
                nc.vector.tensor_add(mha[:, t * C:(t + 1) * C], pm[:, :C], outb[:])

            # ---- stage E: GEMM2 ----
            pf = [g2ps.tile([C, cw], F32, tag=f"g2_{i}", name=f"g2_{i}")
                  for i, (c0, cw) in enumerate(TCH)]
            for ab in range(AB):
                zt = utp.tile([128, C], F32R, tag="zt", name=f"zt_{ab}", bufs=6)
                ci_, t_ = divmod(ab, RT)
                nc.scalar.dma_start(zt[:], zkv_bo[ci_ * 128:(ci_ + 1) * 128,
                                                  t_ * C:(t_ + 1) * C])
                utt = utp.tile([128, R], F32R, tag="ut", name=f"ut_{ab}")
                nc.scalar.dma_start(utt[:], ut_d[ab * 128:(ab + 1) * 128, :])
                for i, (c0, cw) in enumerate(TCH):
                    nc.tensor.matmul(pf[i][:], zt[:],
                                     utt[:, c0:c0 + cw],
                                     start=(ab == 0), stop=(ab == AB - 1))
            furT = ap.tile([C, R], F32)
            for i, (c0, cw) in enumerate(TCH):
                nc.scalar.copy(furT[:, c0:c0 + cw], pf[i][:])
            fur = ap.tile([128, RT * C], F32)
            for t in range(RT):
                ptf = g1ps.tile([128, 128], F32, tag=f"g1_{t % NSUB}", name=f"ftp_{t}")
                nc.tensor.matmul(ptf[:, :C], furT[:, t * 128:(t + 1) * 128],
                                 ident[:C, :C], is_transpose=True)
                nc.vector.tensor_copy(fur[:, t * C:(t + 1) * C], ptf[:, :C])

            # ---- stage G: gate LN (over h|mha|fur), softmax, mix ----
            gm, ginv = _ln_stats(nc, sc2, [h32[:], mha[:], fur[:]], 192, epst, "gln")
            lg_all = sc1.tile([128, RT * 3], F32, tag="lg", name="lg_all")
            catln = sc2.tile([128, 192], F32, tag="catln", name="catln", bufs=2)
            nsc3 = sc2.tile([128, C], F32, tag="nsc", name="nsc_g", bufs=2)
            for t in range(RT):
                _ln_norm(nc, h32[:, t * C:(t + 1) * C], gm[:, t:t + 1], ginv[:, t:t + 1],
                         ggb[:, 0:C], ggb[:, 192:192 + C], catln[:, 0:C], nsc3)
                _ln_norm(nc, mha[:, t * C:(t + 1) * C], gm[:, t:t + 1], ginv[:, t:t + 1],
                         ggb[:, C:2 * C], ggb[:, 192 + C:192 + 2 * C],
                         catln[:, C:2 * C], nsc3)
                _ln_norm(nc, fur[:, t * C:(t + 1) * C], gm[:, t:t + 1], ginv[:, t:t + 1],
                         ggb[:, 2 * C:192], ggb[:, 192 + 2 * C:2 * 192],
                         catln[:, 2 * C:192], nsc3)
                pc1 = g1ps.tile([128, 128], F32, tag="g1_0", name=f"ct1_{t}")
                nc.tensor.matmul(pc1[:, :], catln[:, 0:128], ident[:, :], is_transpose=True)
                ct1 = sc2.tile([128, 128], F32, tag="ct1", name=f"ct1s{t}")
                nc.vector.tensor_copy(ct1[:], pc1[:, :])
                pc2 = g1ps.tile([128, 128], F32, tag="g1_1", name=f"ct2_{t}")
                nc.tensor.matmul(pc2[:C, :], catln[:, 128:192], ident[:, :], is_transpose=True)
                ct2 = sc2.tile([C, 128], F32, tag="ct2", name=f"ct2s{t}")
                nc.vector.tensor_copy(ct2[:], pc2[:C, :])
                pl = g1ps.tile([128, 128], F32, tag="g1_2", name=f"lg_{t}")
                nc.tensor.matmul(pl[:, 0:3], ct1[:], gw1[:], start=True, stop=False)
                nc.tensor.matmul(pl[:, 0:3], ct2[:], gw2[:], start=False, stop=True)
                nc.vector.tensor_add(lg_all[:, t * 3:(t + 1) * 3], pl[:, 0:3], gbias[:])
            # batched softmax over [128, RT, 3]
            lgv = lg_all[:].rearrange("p (t c) -> p t c", t=RT)
            lmax = sc2.tile([128, RT], F32, tag="lmax", name="lmax")
            nc.vector.reduce_max(lmax[:], lgv, axis=AX.X)
            for t in range(RT):
                nc.vector.tensor_scalar(lg_all[:, t * 3:(t + 1) * 3],
                                        lg_all[:, t * 3:(t + 1) * 3],
                                        lmax[:, t:t + 1], None, op0=ALU.subtract)
            nc.scalar.activation(lg_all[:], lg_all[:], ACT.Exp)
            lsum = sc2.tile([128, RT], F32, tag="lsum", name="lsum")
            nc.vector.reduce_sum(lsum[:], lgv, axis=AX.X)
            linv = sc2.tile([128, RT], F32, tag="linv", name="linv")
            nc.vector.reciprocal(linv[:], lsum[:])
            for t in range(RT):
                nc.vector.tensor_scalar(lg_all[:, t * 3:(t + 1) * 3],
                                        lg_all[:, t * 3:(t + 1) * 3],
                                        linv[:, t:t + 1], None, op0=ALU.mult)

            mixs = ap.tile([128, RT * C], F32)
            for t in range(RT):
                mix = mixs[:, t * C:(t + 1) * C]
                mt = sc2.tile([128, C], F32, tag="mixt", name=f"mixt{t}")
                nc.vector.tensor_scalar(mix[:], h32[:, t * C:(t + 1) * C],
                                        lg_all[:, t * 3:t * 3 + 1], None, op0=ALU.mult)
                nc.vector.tensor_scalar(mt[:], mha[:, t * C:(t + 1) * C],
                                        lg_all[:, t * 3 + 1:t * 3 + 2], None, op0=ALU.mult)
                nc.vector.tensor_add(mix[:], mix[:], mt[:])
                nc.vector.tensor_scalar(mt[:], fur[:, t * C:(t + 1) * C],
                                        lg_all[:, t * 3 + 2:t * 3 + 3], None, op0=ALU.mult)
                nc.vector.tensor_add(mix[:], mix[:], mt[:])

            # ---- stage H: FFN + residual -> out ----
            fm, finv = _ln_stats(nc, sc2, [mixs[:]], C, epst, "ffnln")
            fT = ap.tile([C, R], F32)
            nsc4 = sc2.tile([128, C], F32, tag="nsc", name="nsc_f", bufs=2)
            for t in range(RT):
                fln = sc2.tile([128, C], F32, tag="fln", name=f"fln{t}")
                _ln_norm(nc, mixs[:, t * C:(t + 1) * C], fm[:, t:t + 1], finv[:, t:t + 1],
                         fgb[:, 0:C], fgb[:, C:2 * C], fln, nsc4)
                pft = g1ps.tile([128, 128], F32, tag="g1_3", name=f"fT_{t}")
                nc.tensor.matmul(pft[:C, :], fln[:], ident[:, :], is_transpose=True)
                nc.vector.tensor_copy(fT[:, t * 128:(t + 1) * 128], pft[:C, :])
            g1T = ap.tile([C, R], F32)
            for ci, (c0, cw) in enumerate(TCH):
                pg = g1ps.tile([128, 512], F32, tag=f"g1_{ci}", name=f"ffn1_{c0}")
                nc.tensor.matmul(pg[:C, :cw], fw1[:], fT[:, c0:c0 + cw],
                                 start=True, stop=True)
                nc.scalar.activation(g1T[:, c0:c0 + cw], pg[:C, :cw], ACT.Gelu,
                                     bias=fb1[:, 0:1])
            for t in range(RT):
                pf2 = g1ps.tile([128, 128], F32, tag=f"g1_{t % NSUB}", name=f"ffn2_{t}")
                nc.tensor.matmul(pf2[:, :C], g1T[:, t * 128:(t + 1) * 128], fw2[:],
                                 start=True, stop=True)
                ot = sc2.tile([128, C], F32, tag="ot", name=f"ot{t}")
                nc.vector.tensor_add(ot[:], pf2[:, :C], fb2[:])
                nc.vector.tensor_add(ot[:], ot[:], mixs[:, t * C:(t + 1) * C])
                nc.gpsimd.dma_start(out_d[t * 128:(t + 1) * 128, :], ot[:])

    nc.compile()
    return nc


def _host_new_e(e, freq_deltas, freq_bias, readout_w, readout_b, alpha_w):
    e = e.astype(np.float64)
    deltas = np.log1p(np.exp(freq_deltas.astype(np.float64))) + DMIN
    freqs = np.cumsum(deltas) + float(freq_bias)
    freqs = OMEGA * np.tanh(freqs / OMEGA)
    powers = e[:, None] ** np.arange(1, K + 1)
    phase = powers[:, :, None] * freqs
    ns = NF ** 0.5
    rw = readout_w.astype(np.float64)
    eig = (rw[:, 0][None, :]
           + np.einsum('nkf,kf->nk', np.sin(phase) / ns, rw[:, 1:1 + NF])
           + np.einsum('nkf,kf->nk', np.cos(phase) / ns, rw[:, 1 + NF:])
           + readout_b.astype(np.float64)[None, :])
    new_e = eig @ alpha_w.astype(np.float64)
    return new_e[:, 0].astype(np.float32)


def kernel(**inp):
    global LAST_RESULT
    import time as _time
    _t0 = _time.time()
    if "nc" not in _CACHE:
        _CACHE["nc"] = build_nc()
    nc = _CACHE["nc"]
    _t1 = _time.time()

    f32 = np.float32
    e = np.asarray(inp["e"], f32)
    u = np.asarray(inp["u"], f32)
    x = np.asarray(inp["x"], f32)

    new_e = _host_new_e(e, np.asarray(inp["freq_deltas"]), np.asarray(inp["freq_bias"]),
                        np.asarray(inp["readout_w"]), np.asarray(inp["readout_b"]),
                        np.asarray(inp["alpha_w"]))
    new_e_pad = np.zeros(P, f32)
    new_e_pad[:N] = new_e
    newe_t = np.ascontiguousarray(new_e_pad.reshape(AB, 128).T)

    lam1 = float(np.exp(np.sum(np.asarray(inp["lq1"], f32) * np.asarray(inp["lk1"], f32))))
    lam2 = float(np.exp(np.sum(np.asarray(inp["lq2"], f32) * np.asarray(inp["lk2"], f32))))
    lam_full = lam1 - lam2 + LAM_INIT

    def bc(v, n=128):
        return np.ascontiguousarray(np.tile(np.asarray(v, f32)[None, :], (n, 1)))

    def col(v):
        return np.ascontiguousarray(np.asarray(v, f32)[:, None])

    u_pad = np.zeros((P, P), f32)
    u_pad[:N, :N] = u
    x_pad = np.zeros((P, NFEAT), f32)
    x_pad[:N] = x

    packr = np.zeros((128, 4 * HID + C), f32)
    fw1_ = np.asarray(inp["fe_w1"], f32)
    for kk in range(4):
        packr[:, kk * HID:(kk + 1) * HID] = fw1_[kk * 128:(kk + 1) * 128, :]
    packr[:, 4 * HID:4 * HID + C] = np.asarray(inp["fe_w2"], f32)

    p128c = np.zeros((128, 1120), f32)
    p128c[:, 0:128] = bc(np.concatenate([inp["mha_ln_g"], inp["mha_ln_b"]]))
    p128c[:, 128:256] = bc(np.concatenate([(1.0 - LAM_INIT) * np.asarray(inp["attn_ln_g"], f32),
                                           (1.0 - LAM_INIT) * np.asarray(inp["attn_ln_b"], f32)]))
    p128c[:, 256:320] = bc(inp["out_b"])
    p128c[:, 320:704] = bc(np.concatenate([inp["gate_ln_g"], inp["gate_ln_b"]]))
    p128c[:, 704:707] = bc(inp["gate_b"])
    p128c[:, 707:835] = bc(np.concatenate([inp["ffn_ln_g"], inp["ffn_ln_b"]]))
    p128c[:, 835:1027] = bc(np.concatenate([inp["bk1"], inp["bk2"], inp["bv"]]))
    p128c[:, 1047:1048] = col(inp["fe_b1"])
    p128c[:, 1048:1051] = np.asarray(inp["gate_w"], f32)[0:128, :]
    p128c[:, 1051:1115] = bc(inp["ffn_b2"])

    p64 = np.zeros((C, 712), f32)
    p64[:, 0:128] = np.concatenate([inp["wq1"], inp["wq2"]], axis=1)
    p64[:, 128:320] = np.concatenate([inp["wk1"], inp["wk2"], inp["wv"]], axis=1)
    p64[:, 320:322] = np.stack([inp["bq1"], inp["bq2"]], axis=1)
    p64[:, 322:386] = np.asarray(inp["out_w"], f32)
    p64[:, 386:450] = np.asarray(inp["ffn_w1"], f32)
    p64[:, 450:451] = col(inp["ffn_b1"])
    p64[:, 451:515] = np.asarray(inp["ffn_w2"], f32)
    p64[:, 515:516] = col(inp["fe_b2"])
    p64[:, 516:517] = np.full((C, 1), -lam_full, f32)
    p64[:, 517:520] = np.asarray(inp["gate_w"], f32)[128:192, :]

    in_maps = []
    for ci in range(NCORES):
        r0, r1 = ci * R, (ci + 1) * R
        mask = np.zeros((128, RT), f32)
        for t in range(RT):
            base = r0 + t * 128
            nreal = min(max(N - base, 0), 128)
            mask[:nreal, t] = 1.0
        p128 = p128c.copy()
        p128[:, 1027:1037] = mask
        p128[:, 1037:1047] = newe_t[:, ci * RT:(ci + 1) * RT]
        m = {
            "u": np.ascontiguousarray(u_pad[:, r0:r1]),
            "ut": np.ascontiguousarray(u_pad[r0:r1].T),
            "xt": np.ascontiguousarray(x_pad[r0:r1].T),
            "packr": packr,
            "pack128": p128,
            "pack64": p64,
        }
        in_maps.append(m)

    _t2 = _time.time()
    res = run_bass_kernel_spmd(nc, in_maps, list(range(NCORES)))
    _t3 = _time.time()
    print(f"[kernel] build+compile={_t1-_t0:.1f}s hostprep={_t2-_t1:.1f}s run={_t3-_t2:.1f}s")
    LAST_RESULT = res
    out = np.concatenate([res.results[ci]["out"] for ci in range(NCORES)], axis=0)
    return out[:N]


# revision 27
# speedup vs baseline: 1.0738x; 1.0407x over previous
"""NoFoDifformer Trainium2 kernel — 8-core SPMD.

Sharding (v3): 1280 rows of x/h per core; u column-sharded for GEMM1 and
row-sharded(transposed) for GEMM2; exactly TWO collectives, both AllGathers:
  AG1: h (each core's feat-encoder rows -> full h, f32r) before GEMM1.
  GEMM1  utx[:, cols_i] = u[:, cols_i]^T h : exact per-core a-slice; lhsT =
         h-full tiles (f32r), moving = u_col row-band chunks -> utx^T local.
  z_loc = new_e_loc * utx_loc (token-major f32r).
  AG2: z_loc + local k1v/k2v partials -> full z + all partials (summed
       locally). No AllReduce chain: collectives are the expensive barrier.
  GEMM2  h_fur rows_i = u[rows_i,:] (z): lhsT = z (f32r), moving = uT chunks.
new_e (eig encoding) is computed on host (tiny).
Both big GEMMs run in float32r (PE full-rate fp32, ~1.5e-4 rel); all small
GEMMs plain fp32. LayerNorm/softmax statistics are batched across the 10
token tiles ([128, 10] stat rows) to keep DVE/ACT instruction counts low.
"""
import numpy as np

import concourse.bass as bass
import concourse.tile as tile
from concourse import bacc, mybir, masks
from concourse.bass_utils import run_bass_kernel_spmd

F32 = mybir.dt.float32
F32R = mybir.dt.float32r
AX = mybir.AxisListType
ALU = mybir.AluOpType
ACT = mybir.ActivationFunctionType

NCORES = 8
N = 10000
P = 10240            # padded
R = P // NCORES      # 1280 rows per core
RT = R // 128        # 10 token tiles per core
C = 64
NFEAT = 512
HID = 128
K = 10
NF = 16
OMEGA = 50.0
DMIN = 0.25
EPS = 1e-5
LAM_INIT = 0.8 - 0.6 * float(np.exp(-0.3 * 0))   # layer 0 -> 0.2

AB = P // 128        # 80 a-blocks (z)
PB = P // 128        # 80 n row-blocks of full h
NSUB = 4             # tail psum tag rotation
TCH = [(0, 512), (512, 512), (1024, 256)]   # token chunks of the 1280 rows

_CACHE = {}
LAST_RESULT = None


def _ln_stats(nc, sp, pieces, d, epst, name):
    """Batched LN stats over RT token tiles. pieces: APs [128, RT*ci] whose
    free dim is (t, ci). Returns (m, inv) each [128, RT]."""
    s = sp.tile([128, RT], F32, tag="ln_s", name=f"{name}_s")
    q = sp.tile([128, RT], F32, tag="ln_q", name=f"{name}_q")
    tmp = sp.tile([128, RT], F32, tag="ln_tmp", name=f"{name}_tmp")
    sq = sp.tile([128, RT * C], F32, tag="ln_sq", name=f"{name}_sq", bufs=1)
    for i, x in enumerate(pieces):
        ci = x.shape[1] // RT
        xv = x.rearrange("p (t c) -> p t c", t=RT)
        nc.vector.reduce_sum((s if i == 0 else tmp)[:], xv, axis=AX.X)
        if i > 0:
            nc.vector.tensor_add(s[:], s[:], tmp[:])
        nc.vector.tensor_tensor(sq[:, : RT * ci], x[:], x[:], op=ALU.mult)
        sqv = sq[:, : RT * ci].rearrange("p (t c) -> p t c", t=RT)
        nc.vector.reduce_sum((q if i == 0 else tmp)[:], sqv, axis=AX.X)
        if i > 0:
            nc.vector.tensor_add(q[:], q[:], tmp[:])
    m = sp.tile([128, RT], F32, tag="ln_m", name=f"{name}_m")
    nc.vector.tensor_scalar_mul(m[:], s[:], 1.0 / d)
    nc.vector.tensor_scalar_mul(q[:], q[:], 1.0 / d)
    nc.vector.tensor_tensor(tmp[:], m[:], m[:], op=ALU.mult)
    nc.vector.tensor_sub(q[:], q[:], tmp[:])
    std = sp.tile([128, RT], F32, tag="ln_std", name=f"{name}_std")
    nc.scalar.activation(std[:], q[:], ACT.Sqrt, bias=epst[:, 0:1])
    inv = sp.tile([128, RT], F32, tag="ln_inv", name=f"{name}_inv")
    nc.vector.reciprocal(inv[:], std[:])
    return m, inv


def _ln_norm_all(nc, x_all, m, inv, g, b, out_all, ci=C, nt=RT):
    """out = (x - m)*inv*g + b batched over nt tiles via broadcast APs.
    x_all/out_all: [128, nt*ci]; m/inv: [128, nt]; g/b: [128, ci]."""
    xv = x_all.rearrange("p (t c) -> p t c", t=nt)
    ov = out_all.rearrange("p (t c) -> p t c", t=nt)
    mb = m[:].unsqueeze(2).broadcast_to([128, nt, ci])
    ib = inv[:].unsqueeze(2).broadcast_to([128, nt, ci])
    gb = g.unsqueeze(1).broadcast_to([128, nt, ci])
    bb = b.unsqueeze(1).broadcast_to([128, nt, ci])
    nc.vector.tensor_tensor(ov, xv, mb, op=ALU.subtract)
    nc.vector.tensor_tensor(ov, ov, ib, op=ALU.mult)
    nc.vector.tensor_tensor(ov, ov, gb, op=ALU.mult)
    nc.vector.tensor_tensor(ov, ov, bb, op=ALU.add)


def build_nc():
    nc = bacc.Bacc("TRN2", target_bir_lowering=False, debug=False, num_devices=NCORES)

    # ---- I/O ----
    u_d = nc.dram_tensor("u", [P, R], F32R, kind="ExternalInput")   # u[:, cols_i]
    ut_d = nc.dram_tensor("ut", [P, R], F32R, kind="ExternalInput")
    xt_d = nc.dram_tensor("xt", [NFEAT, R], F32R, kind="ExternalInput")
    pr_d = nc.dram_tensor("packr", [128, 4 * HID + C], F32R, kind="ExternalInput")
    p128_d = nc.dram_tensor("pack128", [128, 1120], F32, kind="ExternalInput")
    p64_d = nc.dram_tensor("pack64", [C, 712], F32, kind="ExternalInput")
    out_d = nc.dram_tensor("out", [R, C], F32, kind="ExternalOutput")

    with tile.TileContext(nc) as tc:
        with (
            tc.tile_pool(name="wpool", bufs=1) as wp,
            tc.tile_pool(name="xpool", bufs=1) as xp,
            tc.tile_pool(name="upool", bufs=6) as up,
            tc.tile_pool(name="utpool", bufs=8) as utp,
            tc.tile_pool(name="apool", bufs=1) as ap,
            tc.tile_pool(name="scratch", bufs=1) as sc1,
            tc.tile_pool(name="scratch2", bufs=2) as sc2,
            tc.tile_pool(name="g1ps", bufs=1, space="PSUM") as g1ps,
            tc.tile_pool(name="g2ps", bufs=1, space="PSUM") as g2ps,
            tc.tile_pool(name="auxps", bufs=1, space="PSUM") as auxps,
            tc.tile_pool(name="dram", bufs=1, space="DRAM") as dr,
        ):
            # ---- packed weights/constants: 3 DMAs ----
            xt = xp.tile([128, 4 * R], F32R)
            for kk in range(4):
                nc.sync.dma_start(xt[:, kk * R:(kk + 1) * R], xt_d[kk * 128:(kk + 1) * 128, :])
            packr = wp.tile([128, 4 * HID + C], F32R)
            nc.gpsimd.dma_start(packr[:], pr_d[:])
            p128 = wp.tile([128, 1120], F32)
            nc.gpsimd.dma_start(p128[:], p128_d[:])
            p64 = wp.tile([C, 712], F32)
            nc.gpsimd.dma_start(p64[:], p64_d[:])

            few1 = packr[:, 0:4 * HID]
            few2 = packr[0:HID, 4 * HID:4 * HID + C]
            # p128 cols: mhgb 128 | attgb 128 | outb 64 | ggb 384 | gbias 3 | fgb 128
            #            | bkv 192 | maskt 10 | newe 10 | feb1 1 | gw1 3
            mhgb = p128[:, 0:128]
            attgb = p128[:, 128:256]
            outb = p128[:, 256:320]
            ggb = p128[:, 320:704]
            gbias = p128[:, 704:707]
            fgb = p128[:, 707:835]
            bkv = p128[:, 835:1027]
            maskt = p128[:, 1027:1037]
            newe = p128[:, 1037:1047]
            feb1 = p128[:, 1047:1048]
            gw1 = p128[:, 1048:1051]
            fb2 = p128[:, 1051:1115]
            # p64 cols: wq 128 | wkv 192 | bq 2 | outw 64 | fw1 64 | fb1 1 | fw2 64
            #           | feb2 1 | neglam 1 | gw2 3 | (pad)
            wq = p64[:, 0:128]
            wkv = p64[:, 128:320]
            bq = p64[:, 320:322]
            outw = p64[:, 322:386]
            fw1 = p64[:, 386:450]
            fb1 = p64[:, 450:451]
            fw2 = p64[:, 451:515]
            feb2 = p64[:, 515:516]
            neglam = p64[:, 516:517]
            gw2 = p64[:, 517:520]

            ident = wp.tile([128, 128], F32)
            masks.make_identity(nc, ident[:])
            epst = wp.tile([128, 1], F32)
            nc.vector.memset(epst[:], EPS)

            # ---- stage A: feat encoder -> hT [64, R]; h token-major ----
            hT = ap.tile([C, R], F32)
            for c0, cw in TCH:
                p1 = auxps.tile([128, 512], F32, tag="aux", name=f"fe1_{c0}")
                for kk in range(4):
                    nc.tensor.matmul(p1[:, :cw], few1[:, kk * HID:(kk + 1) * HID],
                                     xt[:, kk * R + c0: kk * R + c0 + cw],
                                     start=(kk == 0), stop=(kk == 3))
                h1 = sc2.tile([128, 512], F32R, tag="h1", name=f"fe1r_{c0}")
                nc.scalar.activation(h1[:, :cw], p1[:, :cw], ACT.Relu, bias=feb1[:, 0:1])
                p2 = auxps.tile([128, 512], F32, tag="aux", name=f"fe2_{c0}")
                nc.tensor.matmul(p2[:C, :cw], few2[:], h1[:, :cw], start=True, stop=True)
                nc.scalar.activation(hT[:, c0:c0 + cw], p2[:C, :cw], ACT.Identity,
                                     bias=feb2[:, 0:1])

            h32 = ap.tile([128, RT * C], F32)
            hr = ap.tile([128, RT * C], F32R)
            for t in range(RT):
                pt = auxps.tile([128, 128], F32, tag="aux", name=f"htp_{t}")
                nc.tensor.matmul(pt[:, :C], hT[:, t * 128:(t + 1) * 128], ident[:C, :C],
                                 is_transpose=True)
                nc.scalar.activation(h32[:, t * C:(t + 1) * C], pt[:, :C], ACT.Copy,
                                     scale=maskt[:, t:t + 1])
                nc.scalar.activation(hr[:, t * C:(t + 1) * C], pt[:, :C], ACT.Copy,
                                     scale=maskt[:, t:t + 1])

            # ---- stage B: mh = LN(h); qT/k/v; k1v,k2v partials + kv AllReduce ----
            mh = ap.tile([128, RT * C], F32)
            mhm, mhinv = _ln_stats(nc, sc2, [h32[:]], C, epst, "mhln")
            _ln_norm_all(nc, h32[:], mhm, mhinv, mhgb[:, 0:C], mhgb[:, C:2 * C], mh[:])

            mhT = ap.tile([C, R], F32)
            for t in range(RT):
                pt = auxps.tile([128, 128], F32, tag="aux", name=f"mhtp_{t}")
                nc.tensor.matmul(pt[:C, :], mh[:, t * C:(t + 1) * C], ident[:, :],
                                 is_transpose=True)
                nc.vector.tensor_copy(mhT[:, t * 128:(t + 1) * 128], pt[:C, :])

            q1T = ap.tile([C, R], F32)
            q2T = ap.tile([C, R], F32)
            for c0, cw in TCH:
                for qi, qT in enumerate([q1T, q2T]):
                    pq = auxps.tile([128, 512], F32, tag="aux", name=f"q{qi}_{c0}")
                    nc.tensor.matmul(pq[:C, :cw], wq[:, qi * C:(qi + 1) * C],
                                     mhT[:, c0:c0 + cw], start=True, stop=True)
                    nc.scalar.activation(qT[:, c0:c0 + cw], pq[:C, :cw], ACT.Identity,
                                         bias=bq[:, qi:qi + 1])

            kv = ap.tile([128, RT * 3 * C], F32)
            for t in range(RT):
                for wi in range(3):
                    pk = auxps.tile([128, 128], F32, tag="aux", name=f"kv{t}_{wi}")
                    nc.tensor.matmul(pk[:, :C], mhT[:, t * 128:(t + 1) * 128],
                                     wkv[:, wi * C:(wi + 1) * C], start=True, stop=True)
                    tkb = sc2.tile([128, C], F32, tag="kvb", name=f"kvb{t}_{wi}")
                    nc.vector.tensor_add(tkb[:], pk[:, :C], bkv[:, wi * C:(wi + 1) * C])
                    nc.scalar.activation(kv[:, (t * 3 + wi) * C:(t * 3 + wi + 1) * C],
                                         tkb[:], ACT.Copy, scale=maskt[:, t:t + 1])

            # k1v/k2v partial accumulate in a bank later reused by GEMM2
            pkv1 = g2ps.tile([128, 512], F32, tag="g2_0", name="pkv1")
            for t in range(RT):
                nc.tensor.matmul(pkv1[:C, 0:C], kv[:, (t * 3) * C:(t * 3 + 1) * C],
                                 kv[:, (t * 3 + 2) * C:(t * 3 + 3) * C],
                                 start=(t == 0), stop=(t == RT - 1))
            for t in range(RT):
                nc.tensor.matmul(pkv1[:C, C:2 * C], kv[:, (t * 3 + 1) * C:(t * 3 + 2) * C],
                                 kv[:, (t * 3 + 2) * C:(t * 3 + 3) * C],
                                 start=(t == 0), stop=(t == RT - 1))
            kvcat = sc1.tile([C, 2 * C], F32, tag="kvcat", name="kvcat")
            nc.vector.tensor_copy(kvcat[:], pkv1[:C, 0:2 * C])

            # ---- AG1: gather full h (f32r); per-core block is SBUF-native ----
            h_bi = dr.tile([128, RT * C], F32R, name="h_bi")
            h_bo = dr.tile([NCORES * 128, RT * C], F32R, name="h_bo", addr_space="Shared")
            nc.gpsimd.dma_start(h_bi[:], hr[:])
            nc.gpsimd.collective_compute(
                "AllGather", ALU.bypass, replica_groups=[list(range(NCORES))],
                ins=[h_bi[:]], outs=[h_bo[:]])


            # ---- stage C: GEMM1 (a-sharded, contraction over all rows) ----
            putx = [g1ps.tile([C, cw], F32, tag=f"g1_{i}", name=f"g1_{i}")
                    for i, (c0, cw) in enumerate(TCH)]
            for nb in range(PB):
                hf = utp.tile([128, C], F32R, tag="hf", name=f"hf_{nb}", bufs=6)
                ci_, t_ = divmod(nb, RT)
                nc.scalar.dma_start(hf[:], h_bo[ci_ * 128:(ci_ + 1) * 128, t_ * C:(t_ + 1) * C])
                uc = up.tile([128, R], F32R, tag="u", name=f"u_{nb}")
                nc.sync.dma_start(uc[:], u_d[nb * 128:(nb + 1) * 128, :])
                for i, (c0, cw) in enumerate(TCH):
                    nc.tensor.matmul(putx[i][:], hf[:],
                                     uc[:, c0:c0 + cw],
                                     start=(nb == 0), stop=(nb == PB - 1))
            utxT = ap.tile([C, R], F32)
            for i, (c0, cw) in enumerate(TCH):
                nc.scalar.copy(utxT[:, c0:c0 + cw], putx[i][:])

            # ---- stage D: z_loc = new_e_loc * utx_loc (token-major f32r) ----
            zloc = ap.tile([128, RT * C], F32R, name="zloc")
            for j in range(RT):
                pz = auxps.tile([128, 128], F32, tag="aux", name=f"zp_{j}")
                nc.tensor.matmul(pz[:, :C], utxT[:, j * 128:(j + 1) * 128],
                                 ident[:C, :C], is_transpose=True)
                nc.scalar.activation(zloc[:, j * C:(j + 1) * C], pz[:, :C],
                                     ACT.Copy, scale=newe[:, j:j + 1])

            # ---- AG2: gather z_loc + kv partials (block = [128, 640+128]) ----
            ZW = RT * C           # 640
            zkv_bi = dr.tile([128, ZW + 128], F32R, name="zkv_bi")
            zkv_bo = dr.tile([NCORES * 128, ZW + 128], F32R, name="zkv_bo",
                             addr_space="Shared")
            nc.gpsimd.dma_start(zkv_bi[:, 0:ZW], zloc[:])
            nc.gpsimd.dma_start(zkv_bi[0:C, ZW:ZW + 128], kvcat[:].bitcast(F32R))
            nc.gpsimd.dma_start(zkv_bi[C:128, ZW:ZW + 128], kvcat[:].bitcast(F32R))
            nc.gpsimd.collective_compute(
                "AllGather", ALU.bypass, replica_groups=[list(range(NCORES))],
                ins=[zkv_bi[:]], outs=[zkv_bo[:]])
            kvg = sc1.tile([C, NCORES * 2 * C], F32, tag="kvg", name="kvg")
            for ci in range(NCORES):
                nc.gpsimd.dma_start(
                    kvg[:, ci * 2 * C:(ci + 1) * 2 * C].bitcast(F32R),
                    zkv_bo[ci * 128:ci * 128 + C, ZW:ZW + 128])
            kvred = wp.tile([C, 2 * C], F32)
            nc.vector.tensor_copy(kvred[:], kvg[:, 0:2 * C])
            for ci in range(1, NCORES):
                nc.vector.tensor_add(kvred[:], kvred[:], kvg[:, ci * 2 * C:(ci + 1) * 2 * C])
            k2vs = wp.tile([C, C], F32)
            nc.scalar.activation(k2vs[:], kvred[:, C:2 * C], ACT.Copy,
                                 scale=neglam[:, 0:1])

            # ---- stage F: xa = q1@k1v - lam*q2@k2v; LN; out proj (early) ----
            xa_all = ap.tile([128, RT * C], F32)
            for t in range(RT):
                pxa = auxps.tile([128, 128], F32, tag="aux", name=f"xa_{t}")
                nc.tensor.matmul(pxa[:, :C], q1T[:, t * 128:(t + 1) * 128],
                                 kvred[:, 0:C], start=True, stop=False)
                nc.tensor.matmul(pxa[:, :C], q2T[:, t * 128:(t + 1) * 128],
                                 k2vs[:], start=False, stop=True)
                nc.vector.tensor_copy(xa_all[:, t * C:(t + 1) * C], pxa[:, :C])
            xam, xainv = _ln_stats(nc, sc2, [xa_all[:]], C, epst, "xaln")
            xaLNT = ap.tile([C, R], F32)
            xaln_all = sc1.tile([128, RT * C], F32, tag="xaln_all", name="xaln_all")
            _ln_norm_all(nc, xa_all[:], xam, xainv, attgb[:, 0:C], attgb[:, C:2 * C],
                         xaln_all[:])
            for t in range(RT):
                pxt = auxps.tile([128, 128], F32, tag="aux", name=f"xat_{t}")
                nc.tensor.matmul(pxt[:C, :], xaln_all[:, t * C:(t + 1) * C],
                                 ident[:, :], is_transpose=True)
                nc.vector.tensor_copy(xaLNT[:, t * 128:(t + 1) * 128], pxt[:C, :])
            mha = ap.tile([128, RT * C], F32)
            for t in range(RT):
                pm = auxps.tile([128, 128], F32, tag="aux", name=f"mha_{t}")
                nc.tensor.matmul(pm[:, :C], xaLNT[:, t * 128:(t + 1) * 128],
                                 outw[:], start=True, stop=True)
                nc.vector.tensor_add(mha[:, t * C:(t + 1) * C], pm[:, :C], outb[:])

            # ---- stage E: GEMM2 ----
            pf = [g2ps.tile([C, cw], F32, tag=f"g2_{i}", name=f"g2_{i}")
                  for i, (c0, cw) in enumerate(TCH)]
            for ab in range(AB):
                zt = utp.tile([128, C], F32R, tag="zt", name=f"zt_{ab}", bufs=6)
                ci_, t_ = divmod(ab, RT)
                nc.sync.dma_start(zt[:], zkv_bo[ci_ * 128:(ci_ + 1) * 128,
                                                t_ * C:(t_ + 1) * C])
                utt = utp.tile([128, R], F32R, tag="ut", name=f"ut_{ab}")
                nc.scalar.dma_start(utt[:], ut_d[ab * 128:(ab + 1) * 128, :])
                for i, (c0, cw) in enumerate(TCH):
                    nc.tensor.matmul(pf[i][:], zt[:],
                                     utt[:, c0:c0 + cw],
                                     start=(ab == 0), stop=(ab == AB - 1))
            furT = ap.tile([C, R], F32)
            for i, (c0, cw) in enumerate(TCH):
                nc.scalar.copy(furT[:, c0:c0 + cw], pf[i][:])
            fur = ap.tile([128, RT * C], F32)
            for t in range(RT):
                ptf = g1ps.tile([128, 128], F32, tag=f"g1_{t % NSUB}", name=f"ftp_{t}")
                nc.tensor.matmul(ptf[:, :C], furT[:, t * 128:(t + 1) * 128],
                                 ident[:C, :C], is_transpose=True)
                nc.vector.tensor_copy(fur[:, t * C:(t + 1) * C], ptf[:, :C])

            # ---- stage G: gate LN (over h|mha|fur), softmax, mix ----
            gm, ginv = _ln_stats(nc, sc2, [h32[:], mha[:], fur[:]], 192, epst, "gln")
            catn = [ap.tile([128, RT * C], F32, name=f"catn{pi}") for pi in range(3)]
            for pi, src in enumerate([h32, mha, fur]):
                _ln_norm_all(nc, src[:], gm, ginv, ggb[:, pi * C:(pi + 1) * C],
                             ggb[:, 192 + pi * C:192 + (pi + 1) * C], catn[pi][:])
            gwsl = [p64[:, 517:520], p64[:, 520:523], p64[:, 523:526]]
            lg_all = sc1.tile([128, RT * 3], F32, tag="lg", name="lg_all")
            for t in range(RT):
                pl = g1ps.tile([128, 128], F32, tag="g1_2", name=f"lg_{t}")
                for pi in range(3):
                    pcx = g1ps.tile([128, 128], F32, tag=f"g1_{pi % 2}", name=f"ct{pi}_{t}")
                    nc.tensor.matmul(pcx[:C, :], catn[pi][:, t * C:(t + 1) * C],
                                     ident[:, :], is_transpose=True)
                    ctx = sc2.tile([C, 128], F32, tag=f"ct{pi}", name=f"ct{pi}s{t}")
                    nc.vector.tensor_copy(ctx[:], pcx[:C, :])
                    nc.tensor.matmul(pl[:, 0:3], ctx[:], gwsl[pi],
                                     start=(pi == 0), stop=(pi == 2))
                nc.vector.tensor_add(lg_all[:, t * 3:(t + 1) * 3], pl[:, 0:3], gbias[:])
            # batched softmax over [128, RT, 3]
            lgv = lg_all[:].rearrange("p (t c) -> p t c", t=RT)
            lmax = sc2.tile([128, RT], F32, tag="lmax", name="lmax")
            nc.vector.reduce_max(lmax[:], lgv, axis=AX.X)
            nc.vector.tensor_tensor(lgv, lgv, lmax[:].unsqueeze(2).broadcast_to(
                [128, RT, 3]), op=ALU.subtract)
            nc.scalar.activation(lg_all[:], lg_all[:], ACT.Exp)
            lsum = sc2.tile([128, RT], F32, tag="lsum", name="lsum")
            nc.vector.reduce_sum(lsum[:], lgv, axis=AX.X)
            linv = sc2.tile([128, RT], F32, tag="linv", name="linv")
            nc.vector.reciprocal(linv[:], lsum[:])
            nc.vector.tensor_tensor(lgv, lgv, linv[:].unsqueeze(2).broadcast_to(
                [128, RT, 3]), op=ALU.mult)

            # mix = h*g0 + mha*g1 + fur*g2 (broadcast over tiles)
            mixs = ap.tile([128, RT * C], F32)
            mixv = mixs[:].rearrange("p (t c) -> p t c", t=RT)
            mtmp = sc1.tile([128, RT * C], F32, tag="mtmp", name="mtmp")
            mtv = mtmp[:].rearrange("p (t c) -> p t c", t=RT)
            def gcol(j):
                return lgv[:, :, j:j + 1].broadcast_to([128, RT, C])
            nc.vector.tensor_tensor(mixv, h32[:].rearrange("p (t c) -> p t c", t=RT),
                                    gcol(0), op=ALU.mult)
            nc.vector.tensor_tensor(mtv, mha[:].rearrange("p (t c) -> p t c", t=RT),
                                    gcol(1), op=ALU.mult)
            nc.vector.tensor_add(mixs[:], mixs[:], mtmp[:])
            nc.vector.tensor_tensor(mtv, fur[:].rearrange("p (t c) -> p t c", t=RT),
                                    gcol(2), op=ALU.mult)
            nc.vector.tensor_add(mixs[:], mixs[:], mtmp[:])

            # ---- stage H: FFN + residual -> out ----
            fm, finv = _ln_stats(nc, sc2, [mixs[:]], C, epst, "ffnln")
            fT = ap.tile([C, R], F32)
            fln_all = sc1.tile([128, RT * C], F32, tag="fln_all", name="fln_all")
            _ln_norm_all(nc, mixs[:], fm, finv, fgb[:, 0:C], fgb[:, C:2 * C], fln_all[:])
            for t in range(RT):
                pft = g1ps.tile([128, 128], F32, tag="g1_3", name=f"fT_{t}")
                nc.tensor.matmul(pft[:C, :], fln_all[:, t * C:(t + 1) * C],
                                 ident[:, :], is_transpose=True)
                nc.vector.tensor_copy(fT[:, t * 128:(t + 1) * 128], pft[:C, :])
            g1T = ap.tile([C, R], F32)
            for ci, (c0, cw) in enumerate(TCH):
                pg = g1ps.tile([128, 512], F32, tag=f"g1_{ci}", name=f"ffn1_{c0}")
                nc.tensor.matmul(pg[:C, :cw], fw1[:], fT[:, c0:c0 + cw],
                                 start=True, stop=True)
                nc.scalar.activation(g1T[:, c0:c0 + cw], pg[:C, :cw], ACT.Gelu,
                                     bias=fb1[:, 0:1])
            for t in range(RT):
                pf2 = g1ps.tile([128, 128], F32, tag=f"g1_{t % NSUB}", name=f"ffn2_{t}")
                nc.tensor.matmul(pf2[:, :C], g1T[:, t * 128:(t + 1) * 128], fw2[:],
                                 start=True, stop=True)
                ot = sc2.tile([128, C], F32, tag="ot", name=f"ot{t}")
                nc.vector.tensor_add(ot[:], pf2[:, :C], fb2[:])
                nc.vector.tensor_add(ot[:], ot[:], mixs[:, t * C:(t + 1) * C])
                nc.gpsimd.dma_start(out_d[t * 128:(t + 1) * 128, :], ot[:])

    nc.compile()
    return nc


def _host_new_e(e, freq_deltas, freq_bias, readout_w, readout_b, alpha_w):
    e = e.astype(np.float64)
    deltas = np.log1p(np.exp(freq_deltas.astype(np.float64))) + DMIN
    freqs = np.cumsum(deltas) + float(freq_bias)
    freqs = OMEGA * np.tanh(freqs / OMEGA)
    powers = e[:, None] ** np.arange(1, K + 1)
    phase = powers[:, :, None] * freqs
    ns = NF ** 0.5
    rw = readout_w.astype(np.float64)
    eig = (rw[:, 0][None, :]
           + np.einsum('nkf,kf->nk', np.sin(phase) / ns, rw[:, 1:1 + NF])
           + np.einsum('nkf,kf->nk', np.cos(phase) / ns, rw[:, 1 + NF:])
           + readout_b.astype(np.float64)[None, :])
    new_e = eig @ alpha_w.astype(np.float64)
    return new_e[:, 0].astype(np.float32)


def kernel(**inp):
    global LAST_RESULT
    import time as _time
    _t0 = _time.time()
    if "nc" not in _CACHE:
        _CACHE["nc"] = build_nc()
    nc = _CACHE["nc"]
    _t1 = _time.time()

    f32 = np.float32
    e = np.asarray(inp["e"], f32)
    u = np.asarray(inp["u"], f32)
    x = np.asarray(inp["x"], f32)

    new_e = _host_new_e(e, np.asarray(inp["freq_deltas"]), np.asarray(inp["freq_bias"]),
                        np.asarray(inp["readout_w"]), np.asarray(inp["readout_b"]),
                        np.asarray(inp["alpha_w"]))
    new_e_pad = np.zeros(P, f32)
    new_e_pad[:N] = new_e
    newe_t = np.ascontiguousarray(new_e_pad.reshape(AB, 128).T)

    lam1 = float(np.exp(np.sum(np.asarray(inp["lq1"], f32) * np.asarray(inp["lk1"], f32))))
    lam2 = float(np.exp(np.sum(np.asarray(inp["lq2"], f32) * np.asarray(inp["lk2"], f32))))
    lam_full = lam1 - lam2 + LAM_INIT

    def bc(v, n=128):
        return np.ascontiguousarray(np.tile(np.asarray(v, f32)[None, :], (n, 1)))

    def col(v):
        return np.ascontiguousarray(np.asarray(v, f32)[:, None])

    u_pad = np.zeros((P, P), f32)
    u_pad[:N, :N] = u
    x_pad = np.zeros((P, NFEAT), f32)
    x_pad[:N] = x

    packr = np.zeros((128, 4 * HID + C), f32)
    fw1_ = np.asarray(inp["fe_w1"], f32)
    for kk in range(4):
        packr[:, kk * HID:(kk + 1) * HID] = fw1_[kk * 128:(kk + 1) * 128, :]
    packr[:, 4 * HID:4 * HID + C] = np.asarray(inp["fe_w2"], f32)

    p128c = np.zeros((128, 1120), f32)
    p128c[:, 0:128] = bc(np.concatenate([inp["mha_ln_g"], inp["mha_ln_b"]]))
    p128c[:, 128:256] = bc(np.concatenate([(1.0 - LAM_INIT) * np.asarray(inp["attn_ln_g"], f32),
                                           (1.0 - LAM_INIT) * np.asarray(inp["attn_ln_b"], f32)]))
    p128c[:, 256:320] = bc(inp["out_b"])
    p128c[:, 320:704] = bc(np.concatenate([inp["gate_ln_g"], inp["gate_ln_b"]]))
    p128c[:, 704:707] = bc(inp["gate_b"])
    p128c[:, 707:835] = bc(np.concatenate([inp["ffn_ln_g"], inp["ffn_ln_b"]]))
    p128c[:, 835:1027] = bc(np.concatenate([inp["bk1"], inp["bk2"], inp["bv"]]))
    p128c[:, 1047:1048] = col(inp["fe_b1"])
    p128c[:, 1048:1051] = np.asarray(inp["gate_w"], f32)[0:128, :]
    p128c[:, 1051:1115] = bc(inp["ffn_b2"])

    p64 = np.zeros((C, 712), f32)
    p64[:, 0:128] = np.concatenate([inp["wq1"], inp["wq2"]], axis=1)
    p64[:, 128:320] = np.concatenate([inp["wk1"], inp["wk2"], inp["wv"]], axis=1)
    p64[:, 320:322] = np.stack([inp["bq1"], inp["bq2"]], axis=1)
    p64[:, 322:386] = np.asarray(inp["out_w"], f32)
    p64[:, 386:450] = np.asarray(inp["ffn_w1"], f32)
    p64[:, 450:451] = col(inp["ffn_b1"])
    p64[:, 451:515] = np.asarray(inp["ffn_w2"], f32)
    p64[:, 515:516] = col(inp["fe_b2"])
    p64[:, 516:517] = np.full((C, 1), -lam_full, f32)
    gw_ = np.asarray(inp["gate_w"], f32)
    p64[:, 517:520] = gw_[0:64, :]
    p64[:, 520:523] = gw_[64:128, :]
    p64[:, 523:526] = gw_[128:192, :]

    in_maps = []
    for ci in range(NCORES):
        r0, r1 = ci * R, (ci + 1) * R
        mask = np.zeros((128, RT), f32)
        for t in range(RT):
            base = r0 + t * 128
            nreal = min(max(N - base, 0), 128)
            mask[:nreal, t] = 1.0
        p128 = p128c.copy()
        p128[:, 1027:1037] = mask
        p128[:, 1037:1047] = newe_t[:, ci * RT:(ci + 1) * RT]
        m = {
            "u": np.ascontiguousarray(u_pad[:, r0:r1]),
            "ut": np.ascontiguousarray(u_pad[r0:r1].T),
            "xt": np.ascontiguousarray(x_pad[r0:r1].T),
            "packr": packr,
            "pack128": p128,
            "pack64": p64,
        }
        in_maps.append(m)

    _t2 = _time.time()
    res = run_bass_kernel_spmd(nc, in_maps, list(range(NCORES)))
    _t3 = _time.time()
    print(f"[kernel] build+compile={_t1-_t0:.1f}s hostprep={_t2-_t1:.1f}s run={_t3-_t2:.1f}s")
    LAST_RESULT = res
    out = np.concatenate([res.results[ci]["out"] for ci in range(NCORES)], axis=0)
    return out[:N]
